# revision 18
# baseline (speedup 1.0000x reference)
"""DAWN layer on 8 trn2 NeuronCores.

Sharding:
- Phase A (input neurons + lateral attention): token-parallel. Core c owns
  128 tokens (batch c//4, s in [(c%4)*128, +128)). Each core receives its
  batch's x ROLLED so its own tokens sit at rows [384:512] (uniform SPMD
  program; causality lives in a per-core additive mask in rolled key order).
- One AllGather moves x1 (pre-transposed to H-major blocks) + enriched_acts
  to every core.
- Phase C (process neurons): expert-parallel. Core c owns experts
  [16c,16c+16) of down_proj/up_proj; conv weights are per-core permuted so
  its experts are rows 0:16 of the conv output. Partial outputs are
  ReduceScattered; each core runs the final LayerNorm on its own tokens.
- Matmuls use float32r (full-rate on TensorE, ~1e-4 rounding).

Host code does data movement only (roll / permute / transpose / reshape).
"""

import contextlib

import numpy as np

import concourse.bass as bass
import concourse.mybir as mybir
import concourse.tile as tile
from concourse import bacc, masks
from concourse.bass_utils import run_bass_kernel_spmd

dt = mybir.dt
AF = mybir.ActivationFunctionType
ALU = mybir.AluOpType
AX = mybir.AxisListType

B, S, H = 2, 512, 768
N_IN, N_PROC, AR, PR = 64, 128, 16, 128
NEG_INF = -1e30
NCORE = 8
TPC = 128               # tokens per core
EPC = N_PROC // NCORE   # experts per core = 16
NR1 = N_IN * AR         # 1024
NR2 = EPC * PR          # 2048
HT = H // 128           # 6
F32 = dt.float32
F32R = dt.float32r
AGW = H + N_IN          # all-gather payload columns (832)


def _f32r(ap):
    return ap.bitcast(F32R)


def build_kernel():
    nc = bacc.Bacc(None)

    def I(name, shape):
        return nc.dram_tensor(name, list(shape), F32, kind="ExternalInput")

    x_roll = I("x_roll", (S, H))
    x_roll_t = I("x_roll_t", (H, S))
    msk = I("msk", (TPC, S))
    patterns = I("patterns", (N_IN, H))
    ws_t = I("ws_t", (H, H))              # w_shared.T -> [h, o]
    b_shared = I("b_shared", (H,))
    adl = I("adl", (H, NR1))              # adapt_down as [h, n*AR+r]
    upa = I("upa", (NR1, H))              # adapt_up as [(n,r), h]
    g1 = I("g1", (H,)); b1 = I("b1", (H,))
    g2 = I("g2", (H,)); b2 = I("b2", (H,))
    wq_t = I("wq_t", (N_IN, N_IN)); wk_t = I("wk_t", (N_IN, N_IN))
    wv_t = I("wv_t", (N_IN, N_IN)); wo_t = I("wo_t", (N_IN, N_IN))
    bq = I("bq", (N_IN,)); bk = I("bk", (N_IN,))
    bv = I("bv", (N_IN,)); bo = I("bo", (N_IN,))
    cw = I("cw", (N_IN, 5, N_PROC))       # conv w [n, ds, o], o permuted per core
    cb = I("cb", (N_PROC,))               # conv bias, permuted per core
    dp = I("dp", (H, NR2))                # down_proj slice [h, e*PR+r]
    up = I("up", (NR2, H))                # up_proj slice [e*PR+r, h]

    def O(name, shape):
        return nc.dram_tensor(name, list(shape), F32, kind="ExternalOutput")

    o_x2 = O("o_x2", (TPC, H))
    o_ia = O("o_ia", (TPC, N_IN))
    o_rel = O("o_rel", (TPC, N_IN))
    o_enr = O("o_enr", (TPC, N_IN))
    o_pacts = O("o_pacts", (N_PROC, B * S))  # both batches, permuted experts
    o_att = O("o_att", (TPC, S))             # rolled key order

    with tile.TileContext(nc) as tc, contextlib.ExitStack() as es:
        dram = es.enter_context(tc.tile_pool(name="dram", bufs=1, space="DRAM"))
        persist = es.enter_context(tc.tile_pool(name="persist", bufs=1))
        bigw = es.enter_context(tc.tile_pool(name="bigw", bufs=1))
        psT = es.enter_context(tc.tile_pool(name="psT", bufs=2, space="PSUM"))

        ag1_in = dram.tile([TPC, N_IN], F32, name="ag1_in", tag="ag1_in")
        ag1_out = dram.tile([NCORE * TPC, N_IN], F32, name="ag1_out",
                            tag="ag1_out", addr_space="Shared")
        ag2_in = dram.tile([TPC, H], F32, name="ag2_in", tag="ag2_in")
        ag2_out = dram.tile([NCORE * TPC, H], F32, name="ag2_out",
                            tag="ag2_out", addr_space="Shared")
        rs_in = dram.tile([NCORE * TPC, H], dt.bfloat16, name="rs_in", tag="rs_in")
        rs_out = dram.tile([TPC, H], dt.bfloat16, name="rs_out", tag="rs_out")
        pacts_d = [dram.tile([N_PROC, S], F32, name=f"pacts_d{b}", tag=f"pacts_d{b}")
                   for b in range(B)]

        # phase-C weights: dedicated space, loads can start immediately
        DP = [bigw.tile([128, NR2], F32R, name=f"dp{i}", tag=f"dp{i}") for i in range(HT)]
        ident = persist.tile([128, 128], F32, name="ident", tag="ident")
        masks.make_identity(nc, ident[:])
        identr = persist.tile([128, 128], F32R, name="identr", tag="identr")
        nc.vector.tensor_copy(identr[:], ident[:])

        def transpose_f32(src_ap):
            """PE-transpose src [p, q] (f32) -> psum tile [q, p]."""
            p, q = src_ap.shape[0], src_ap.shape[-1]
            t = psT.tile([q, p], F32, name="tp", tag="tp")
            nc.tensor.transpose(t[:], src_ap, ident[:p, :p])
            return t

        def brow(pool, name, src, n):
            t = pool.tile([128, n], F32, name=name, tag=name)
            nc.gpsimd.dma_start(out=t, in_=src[:].partition_broadcast(128))
            return t

        BVB = brow(persist, "BVB", bv, N_IN); BOB = brow(persist, "BOB", bo, N_IN)

        def pcol(name, src, n):
            t = persist.tile([n, 1], F32, tag=name)
            nc.sync.dma_start(out=t, in_=src[:].unsqueeze(-1))
            return t

        BQ = pcol("BQ", bq, N_IN); BK = pcol("BK", bk, N_IN)
        CBT = pcol("CBT", cb, N_PROC)

        EPS = persist.tile([128, 1], F32, name="EPS", tag="EPS")
        nc.vector.memset(EPS, 1e-5)
        C8 = persist.tile([128, 1], F32, name="C8", tag="C8")
        nc.vector.memset(C8, 1.0 / float(np.sqrt(N_IN)))

        X1G = persist.tile([TPC, H], F32, name="X1G", tag="X1G")       # own x1

        def layernorm(pool, dst, src, gb, bb):
            st = pool.tile([128, 3, nc.vector.BN_STATS_DIM], F32, name="lnst", tag="lnst")
            s3 = src.rearrange("p (a b) -> p a b", a=3)
            for a in range(3):
                nc.vector.bn_stats(out=st[:, a, :], in_=s3[:, a, :])
            mv = pool.tile([128, nc.vector.BN_AGGR_DIM], F32, name="lnmv", tag="lnmv")
            nc.vector.bn_aggr(out=mv[:], in_=st[:])
            rstd = pool.tile([128, 1], F32, name="lnrstd", tag="lnrstd")
            nc.scalar.activation(rstd[:], mv[:, 1:2], AF.Ln, bias=EPS[:])
            nc.scalar.activation(rstd[:], rstd[:], AF.Exp, scale=-0.5)
            nc.vector.tensor_scalar(out=dst, in0=src, scalar1=mv[:, 0:1],
                                    scalar2=rstd[:], op0=ALU.subtract,
                                    op1=ALU.mult)
            nc.vector.tensor_mul(dst, dst, gb[:])
            nc.vector.tensor_add(dst, dst, bb[:])

        # ================= PHASE A =================
        with contextlib.ExitStack() as esA:
            wsp = esA.enter_context(tc.tile_pool(name="wsp", bufs=2))
            xbp = esA.enter_context(tc.tile_pool(name="xbp", bufs=2))
            xh = esA.enter_context(tc.tile_pool(name="xh", bufs=1))
            sb = esA.enter_context(tc.tile_pool(name="sb", bufs=2))
            att = esA.enter_context(tc.tile_pool(name="att", bufs=1))
            psA = esA.enter_context(tc.tile_pool(name="psA", bufs=6, space="PSUM"))

            BSH = brow(att, "BSH", b_shared, H)
            G1B = brow(att, "G1B", g1, H); B1B = brow(att, "B1B", b1, H)

            # ---- x: H-major direct loads + per-block l2 stats ----
            XH = [xh.tile([128, S], F32R, name=f"XH{i}", tag=f"XH{i}")
                  for i in range(HT)]
            for i in range(HT):
                nc.sync.dma_start(out=XH[i],
                                  in_=_f32r(x_roll_t[128 * i:128 * (i + 1), :]))
            RNt = [att.tile([128, 1], F32, name=f"RN{t}", tag=f"RN{t}")
                   for t in range(4)]
            XB3 = att.tile([128, H], F32, name="XB3", tag="XB3")
            ENRO = att.tile([TPC, N_IN], F32, name="ENRO", tag="ENRO")
            for t in range(4):
                xb = xbp.tile([128, H], F32, name="xb", tag="xb")
                nc.sync.dma_start(out=xb, in_=x_roll[128 * t:128 * (t + 1), :])
                sq = sb.tile([128, H], F32, name="sq", tag="sq")
                nc.vector.tensor_mul(sq[:], xb[:], xb[:])
                ss = sb.tile([128, 1], F32, name="ss", tag="ss")
                nc.vector.reduce_sum(ss[:], sq[:], axis=AX.X)
                nc.scalar.activation(ss[:], ss[:], AF.Ln)
                nc.scalar.activation(RNt[t][:], ss[:], AF.Exp, scale=-0.5)
                if t == 3:
                    nc.vector.tensor_copy(XB3[:], xb[:])

            # ---- patterns: l2-normalize rows, transpose to [h, n] ----
            PT = sb.tile([N_IN, H], F32, name="pt", tag="pt", bufs=1)
            nc.sync.dma_start(out=PT, in_=patterns[:])
            psq = sb.tile([N_IN, H], F32, name="psq", tag="psq", bufs=1)
            nc.vector.tensor_mul(psq[:], PT[:], PT[:])
            prs = sb.tile([N_IN, 1], F32, name="prs", tag="prs")
            nc.vector.reduce_sum(prs[:], psq[:], axis=AX.X)
            nc.scalar.activation(prs[:], prs[:], AF.Ln)
            nc.scalar.activation(prs[:], prs[:], AF.Exp, scale=-0.5)
            nc.vector.tensor_scalar_mul(out=PT[:], in0=PT[:], scalar1=prs[:])
            PNH = [att.tile([128, N_IN], F32R, name=f"PNH{i}", tag=f"PNH{i}")
                   for i in range(HT)]
            for i in range(HT):
                tp = transpose_f32(PT[:, 128 * i:128 * (i + 1)])
                nc.vector.tensor_copy(PNH[i][:], tp[:])

            # ---- input_acts for all 512 tokens ----
            ACTS = att.tile([128, 4 * N_IN], F32, name="ACTS", tag="ACTS")
            for t in range(4):
                rp = psA.tile([128, 512], F32, name="ps", tag="ps")
                for i in range(HT):
                    nc.tensor.matmul(rp[:, :N_IN],
                                     XH[i][:, 128 * t:128 * (t + 1)], PNH[i][:],
                                     start=(i == 0), stop=(i == HT - 1))
                nc.scalar.activation(ACTS[:, N_IN * t:N_IN * (t + 1)],
                                     rp[:, :N_IN], AF.Sigmoid, scale=RNt[t][:])
            AOWN = ACTS[:, N_IN * 3:N_IN * 4]
            nc.sync.dma_start(out=o_ia[:], in_=AOWN)

            ANT = att.tile([N_IN, S], F32R, name="ANT", tag="ANT")
            for t in range(4):
                tp = transpose_f32(ACTS[:, N_IN * t:N_IN * (t + 1)])
                nc.vector.tensor_copy(ANT[:, 128 * t:128 * (t + 1)], tp[:])

            # ---- lateral attention (own 128 queries, all 512 keys) ----
            WQ = att.tile([N_IN, N_IN], F32R, name="WQ", tag="WQ")
            WK = att.tile([N_IN, N_IN], F32R, name="WK", tag="WK")
            WV = att.tile([N_IN, N_IN], F32R, name="WV", tag="WV")
            WO = att.tile([N_IN, N_IN], F32R, name="WO", tag="WO")
            nc.sync.dma_start(out=WQ, in_=_f32r(wq_t[:]))
            nc.sync.dma_start(out=WK, in_=_f32r(wk_t[:]))
            nc.sync.dma_start(out=WV, in_=_f32r(wv_t[:]))
            nc.sync.dma_start(out=WO, in_=_f32r(wo_t[:]))
            MT = att.tile([128, S], F32, name="MT", tag="MT")
            nc.sync.dma_start(out=MT, in_=msk[:])

            qp = psA.tile([128, 512], F32, name="ps", tag="ps")
            nc.tensor.matmul(qp[:N_IN, :TPC], WQ[:], ANT[:, 384:512],
                             start=True, stop=True)
            QM = att.tile([N_IN, TPC], F32R, name="QM", tag="QM")
            nc.vector.tensor_scalar(out=QM[:], in0=qp[:N_IN, :TPC],
                                    scalar1=BQ[:], scalar2=None, op0=ALU.add)
            kp = psA.tile([128, 512], F32, name="ps", tag="ps")
            nc.tensor.matmul(kp[:N_IN, :], WK[:], ANT[:], start=True, stop=True)
            KM = att.tile([N_IN, S], F32R, name="KM", tag="KM")
            nc.vector.tensor_scalar(out=KM[:], in0=kp[:N_IN, :],
                                    scalar1=BK[:], scalar2=None, op0=ALU.add)
            VT = att.tile([128, 4 * N_IN], F32R, name="VT", tag="VT")
            for t in range(4):
                vp = psA.tile([128, 512], F32, name="ps", tag="ps")
                nc.tensor.matmul(vp[:, :N_IN], ANT[:, 128 * t:128 * (t + 1)],
                                 WV[:], start=True, stop=True)
                nc.vector.tensor_add(VT[:, N_IN * t:N_IN * (t + 1)],
                                     vp[:, :N_IN], BVB[:])
            scp = psA.tile([128, 512], F32, name="ps", tag="ps")
            nc.tensor.matmul(scp[:], QM[:], KM[:], start=True, stop=True)
            SCM = att.tile([128, S], F32, name="SCM", tag="SCM")
            nc.vector.tensor_scalar_mul(out=SCM[:], in0=scp[:], scalar1=C8[:])
            nc.vector.tensor_add(SCM[:], SCM[:], MT[:])
            NM = sb.tile([128, 1], F32, name="NM", tag="NM")
            nc.vector.reduce_max(NM[:], SCM[:], axis=AX.X, negate=True)
            ATT = att.tile([128, S], F32, name="ATT", tag="ATT")
            nc.scalar.activation(ATT[:], SCM[:], AF.Exp, bias=NM[:])
            DN = sb.tile([128, 1], F32, name="DN", tag="DN")
            nc.vector.reduce_sum(DN[:], ATT[:], axis=AX.X)
            nc.vector.reciprocal(DN[:], DN[:])
            nc.vector.tensor_scalar_mul(out=ATT[:], in0=ATT[:], scalar1=DN[:])
            nc.sync.dma_start(out=o_att[:], in_=ATT[:])
            ATK = [att.tile([128, 128], F32R, name=f"ATK{t}", tag=f"ATK{t}")
                   for t in range(4)]
            for t in range(4):
                tp = transpose_f32(ATT[:, 128 * t:128 * (t + 1)])
                nc.vector.tensor_copy(ATK[t][:], tp[:])
            ctxp = psA.tile([128, 512], F32, name="ps", tag="ps")
            for t in range(4):
                nc.tensor.matmul(ctxp[:, :N_IN], ATK[t][:],
                                 VT[:, N_IN * t:N_IN * (t + 1)],
                                 start=(t == 0), stop=(t == 3))
            CTX = sb.tile([128, N_IN], F32, name="CTX", tag="CTX")
            nc.vector.tensor_copy(CTX[:], ctxp[:, :N_IN])
            tp = transpose_f32(CTX[:])
            CTN = sb.tile([N_IN, TPC], F32R, name="CTN", tag="CTN")
            nc.vector.tensor_copy(CTN[:], tp[:])
            relp = psA.tile([128, 512], F32, name="ps", tag="ps")
            nc.tensor.matmul(relp[:, :N_IN], CTN[:], WO[:], start=True, stop=True)
            RELS = sb.tile([TPC, N_IN], F32, name="RELS", tag="RELS")
            nc.vector.tensor_add(RELS[:], relp[:, :N_IN], BOB[:])
            nc.sync.dma_start(out=o_rel[:], in_=RELS[:])
            nc.vector.tensor_add(ENRO[:], RELS[:], AOWN)
            nc.sync.dma_start(out=o_enr[:], in_=ENRO[:])
            nc.sync.dma_start(out=ag1_in[:], in_=ENRO[:])

            # enriched all-gather fires while the shared/adapter path runs
            nc.gpsimd.collective_compute(
                "AllGather", ALU.bypass,
                replica_groups=[list(range(NCORE))],
                ins=[ag1_in[:].opt()], outs=[ag1_out[:].opt()],
            )

            # ---- shared = gelu(x @ Wsh + b) (own tokens) ----
            SHP = [psA.tile([128, 512], F32, name="ps", tag="ps") for _ in range(2)]
            for i in range(HT):
                wst = wsp.tile([128, H], F32R, name="ws", tag="ws")
                nc.scalar.dma_start(out=wst,
                                    in_=_f32r(ws_t[128 * i:128 * (i + 1), :]))
                for h2 in range(2):
                    nc.tensor.matmul(SHP[h2][:, :384], XH[i][:, 384:512],
                                     wst[:, 384 * h2:384 * (h2 + 1)],
                                     start=(i == 0), stop=(i == HT - 1))
            SHG = sb.tile([128, H], F32, name="SHG", tag="SHG", bufs=1)
            for h2 in range(2):
                nc.vector.tensor_add(SHG[:, 384 * h2:384 * (h2 + 1)],
                                     SHP[h2][:, :384],
                                     BSH[:, 384 * h2:384 * (h2 + 1)])
            nc.scalar.activation(SHG[:], SHG[:], AF.Gelu)
            SHH = [att.tile([128, 128], F32R, name=f"SHH{i}", tag=f"SHH{i}")
                   for i in range(HT)]
            for i in range(HT):
                tp = transpose_f32(SHG[:, 128 * i:128 * (i + 1)])
                nc.vector.tensor_copy(SHH[i][:], tp[:])

            # ---- adapters: down -> scale by acts -> spec ----
            SDT = sb.tile([128, NR1], F32R, name="SDT", tag="SDT", bufs=1)
            ADT = [wsp.tile([128, NR1], F32R, name=f"adl{i}", tag=f"adl{i}",
                            bufs=1) for i in range(HT)]
            for i in range(HT):
                nc.scalar.dma_start(out=ADT[i],
                                    in_=_f32r(adl[128 * i:128 * (i + 1), :]))
            for nh in range(2):
                dpp = psA.tile([128, 512], F32, name="ps", tag="ps")
                for i in range(HT):
                    nc.tensor.matmul(dpp[:], SHH[i][:],
                                     ADT[i][:, 512 * nh:512 * (nh + 1)],
                                     start=(i == 0), stop=(i == HT - 1))
                ab = ACTS[:, N_IN * 3 + 32 * nh:N_IN * 3 + 32 * (nh + 1)]
                abc = bass.AP(tensor=ab.tensor, offset=ab.offset,
                              ap=[list(ab.ap[0]), [ab.ap[1][0], 32], [0, AR]])
                sdt3 = SDT[:, 512 * nh:512 * (nh + 1)].rearrange(
                    "p (a b) -> p a b", a=32)
                dpp3 = dpp[:].rearrange("p (a b) -> p a b", a=32)
                nc.vector.tensor_tensor(out=sdt3, in0=dpp3, in1=abc, op=ALU.mult)
            SDN = [att.tile([128, 128], F32R, name=f"SDN{j}", tag=f"SDN{j}")
                   for j in range(8)]
            for j in range(8):
                t = psT.tile([128, 128], F32, name="tp", tag="tp")
                nc.tensor.transpose(t[:].bitcast(F32R),
                                    SDT[:, 128 * j:128 * (j + 1)], identr[:])
                nc.vector.tensor_copy(SDN[j][:], t[:].bitcast(F32R))
            SPP = [psA.tile([128, 512], F32, name="ps", tag="ps") for _ in range(2)]
            for j in range(8):
                upt = wsp.tile([128, H], F32R, name="upa", tag="upa")
                nc.scalar.dma_start(out=upt,
                                    in_=_f32r(upa[128 * j:128 * (j + 1), :]))
                for h2 in range(2):
                    nc.tensor.matmul(SPP[h2][:, :384], SDN[j][:],
                                     upt[:, 384 * h2:384 * (h2 + 1)],
                                     start=(j == 0), stop=(j == 7))

            # ---- intermediate + LN1 -> x1 ----
            SA = sb.tile([128, 1], F32, name="SA", tag="SA")
            nc.vector.reduce_sum(SA[:], AOWN, axis=AX.X)
            XI = sb.tile([128, H], F32, name="XI", tag="XI", bufs=1)
            nc.vector.tensor_scalar_mul(out=XI[:], in0=SHG[:], scalar1=SA[:])
            for h2 in range(2):
                nc.vector.tensor_add(XI[:, 384 * h2:384 * (h2 + 1)],
                                     XI[:, 384 * h2:384 * (h2 + 1)],
                                     SPP[h2][:, :384])
            nc.vector.tensor_add(XI[:], XI[:], XB3[:])
            layernorm(sb, X1G[:], XI[:], G1B, B1B)

            # pack x1 H-major into ag2_in
            for i in range(HT):
                tp = transpose_f32(X1G[:, 128 * i:128 * (i + 1)])
                xhc = sb.tile([128, 128], F32, name="xhc", tag="xhc")
                nc.vector.tensor_copy(xhc[:], tp[:])
                nc.sync.dma_start(out=ag2_in[:, 128 * i:128 * (i + 1)], in_=xhc[:])

        # ================= ALLGATHER (x1) =================
        nc.gpsimd.collective_compute(
            "AllGather", ALU.bypass,
            replica_groups=[list(range(NCORE))],
            ins=[ag2_in[:].opt()], outs=[ag2_out[:].opt()],
        )

        # ================= PHASE C =================
        with contextlib.ExitStack() as esC:
            cp = esC.enter_context(tc.tile_pool(name="cp", bufs=2))
            x1p = esC.enter_context(tc.tile_pool(name="x1p", bufs=1))
            sdp = esC.enter_context(tc.tile_pool(name="sdp", bufs=1))
            upp = esC.enter_context(tc.tile_pool(name="upp", bufs=1))
            bc = esC.enter_context(tc.tile_pool(name="bc", bufs=2))
            psC = esC.enter_context(tc.tile_pool(name="psC", bufs=2, space="PSUM"))
            psY = esC.enter_context(tc.tile_pool(name="psY", bufs=4, space="PSUM"))

            # big weights now; DP tiles were reserved up-front
            for i in range(HT):
                nc.gpsimd.dma_start(out=DP[i],
                                    in_=_f32r(dp[128 * i:128 * (i + 1), :]))
            UP = [upp.tile([128, H], F32R, name=f"up{e}", tag=f"up{e}")
                  for e in range(EPC)]
            for e in range(EPC):
                nc.gpsimd.dma_start(out=UP[e],
                                    in_=_f32r(up[128 * e:128 * (e + 1), :]))

            PACTS = [x1p.tile([N_PROC, S], F32, name=f"PACTS{b}", tag=f"PACTS{b}")
                     for b in range(B)]
            G2B = brow(x1p, "G2B", g2, H); B2B = brow(x1p, "B2B", b2, H)

            # conv over gathered enriched -> process_acts^T per batch
            CWT = x1p.tile([N_IN, 5 * N_PROC], F32R, name="CWT", tag="CWT")
            nc.sync.dma_start(out=CWT,
                              in_=_f32r(cw[:].rearrange("n a o -> n (a o)")))
            ENT = [x1p.tile([N_IN, S + 4], F32R, name=f"ENT{b}", tag=f"ENT{b}")
                   for b in range(B)]
            ZC = x1p.tile([N_IN, 2], F32, name="ZC", tag="ZC")
            nc.vector.memset(ZC, 0.0)
            for b in range(B):
                nc.vector.tensor_copy(ENT[b][:, 0:2], ZC[:])
                nc.vector.tensor_copy(ENT[b][:, S + 2:S + 4], ZC[:])
            for c in range(NCORE):
                b, blk = c // 4, c % 4
                ec = cp.tile([TPC, N_IN], F32, name="ec", tag="ec")
                nc.sync.dma_start(out=ec, in_=ag1_out[TPC * c:TPC * (c + 1), :])
                tp = transpose_f32(ec[:])
                nc.vector.tensor_copy(
                    ENT[b][:, 2 + 128 * blk:2 + 128 * (blk + 1)], tp[:])
            for b in range(B):
                rsp = psC.tile([N_PROC, S], F32, name="pd", tag="pd")
                for ds in range(5):
                    nc.tensor.matmul(rsp[:], CWT[:, N_PROC * ds:N_PROC * (ds + 1)],
                                     ENT[b][:, ds:ds + S],
                                     start=(ds == 0), stop=(ds == 4))
                nc.scalar.activation(PACTS[b][:], rsp[:], AF.Sigmoid, bias=CBT[:])
                nc.sync.dma_start(out=o_pacts[:, S * b:S * (b + 1)],
                                  in_=PACTS[b][:])
                nc.sync.dma_start(out=pacts_d[b][:], in_=PACTS[b][:])

            # PE warm-keeper: chained f32 matmuls on x1 bridging the AG2 wait
            wup = psC.tile([128, 512], F32, name="wup", tag="pd")
            for w in range(8):
                nc.tensor.matmul(wup[:], X1G[:, 0:128], X1G[:, 0:512],
                                 start=(w == 0), stop=(w == 7))

            SD = [sdp.tile([128, S], F32R, name=f"SD{e}", tag=f"SD{e}")
                  for e in range(EPC)]
            for b in range(B):
                # x1 H-major (this batch) from the gathered payload
                X1H = [x1p.tile([128, S], F32R, name=f"X1H{i}", tag=f"X1H{i}")
                       for i in range(HT)]
                for i in range(HT):
                    blk = ag2_out[S * b:S * (b + 1), 128 * i:128 * (i + 1)]
                    src_ap = blk.rearrange("(c t) h -> h c t", c=4)
                    nc.sync.dma_start(
                        out=X1H[i][:].rearrange("h (c t) -> h c t", c=4),
                        in_=_f32r(src_ap))
                # pass 1: down-proj + gate by process_acts
                for e in range(EPC):
                    pd = psC.tile([PR, S], F32, name="pd", tag="pd")
                    for i in range(HT):
                        nc.tensor.matmul(pd[:], DP[i][:, PR * e:PR * (e + 1)],
                                         X1H[i][:],
                                         start=(i == 0), stop=(i == HT - 1))
                    bce = bc.tile([PR, S], F32, name="bce", tag="bce")
                    nc.gpsimd.dma_start(
                        out=bce, in_=pacts_d[b][e].partition_broadcast(PR))
                    nc.vector.tensor_tensor(out=SD[e][:], in0=pd[:], in1=bce[:],
                                            op=ALU.mult)
                # pass 2: up-proj, accumulate over experts in PSUM
                for tb in range(4):
                    yp = [psY.tile([128, 384], F32, name="yp", tag="yp")
                          for _ in range(2)]
                    for e in range(EPC):
                        for h2 in range(2):
                            nc.tensor.matmul(yp[h2][:],
                                             SD[e][:, 128 * tb:128 * (tb + 1)],
                                             UP[e][:, 384 * h2:384 * (h2 + 1)],
                                             start=(e == 0), stop=(e == EPC - 1))
                    ys = cp.tile([128, H], dt.bfloat16, name="ys", tag="ys")
                    for h2 in range(2):
                        nc.vector.tensor_copy(ys[:, 384 * h2:384 * (h2 + 1)],
                                              yp[h2][:])
                    nc.sync.dma_start(
                        out=rs_in[S * b + 128 * tb:S * b + 128 * (tb + 1), :],
                        in_=ys[:])

            nc.gpsimd.collective_compute(
                "ReduceScatter", ALU.add,
                replica_groups=[list(range(NCORE))],
                ins=[rs_in[:].opt()], outs=[rs_out[:].opt()],
            )

            YRb = cp.tile([TPC, H], dt.bfloat16, name="YRb", tag="YRb", bufs=1)
            nc.sync.dma_start(out=YRb, in_=rs_out[:])
            YR = cp.tile([TPC, H], F32, name="YR", tag="YR", bufs=1)
            nc.vector.tensor_add(YR[:], YRb[:], X1G[:])
            layernorm(cp, YR[:], YR[:], G2B, B2B)
            nc.sync.dma_start(out=o_x2[:], in_=YR[:])

    nc.finalize()
    return nc


_NC_CACHE = None


def _get_nc():
    global _NC_CACHE
    if _NC_CACHE is None:
        _NC_CACHE = build_kernel()
    return _NC_CACHE


def _prep_inputs(inputs):
    f = lambda k: np.asarray(inputs[k], dtype=np.float32)
    x = f("x"); patterns = f("patterns")
    w_shared = f("w_shared"); b_shared = f("b_shared")
    adapt_down = f("adapt_down"); adapt_up = f("adapt_up")
    g1 = f("g1"); b1 = f("b1"); g2 = f("g2"); b2 = f("b2")
    in_proj_w = f("in_proj_w"); in_proj_b = f("in_proj_b")
    out_proj_w = f("out_proj_w"); out_proj_b = f("out_proj_b")
    conv_w = f("conv_w"); conv_b = f("conv_b")
    down_proj = f("down_proj"); up_proj = f("up_proj")

    adl = np.ascontiguousarray(adapt_down.transpose(1, 0, 2).reshape(H, NR1))
    upa = np.ascontiguousarray(adapt_up.reshape(NR1, H))
    ws_t = np.ascontiguousarray(w_shared.T)  # 'bsh,oh->bso': rows of w are o
    wq_t = np.ascontiguousarray(in_proj_w[0:N_IN].T)
    wk_t = np.ascontiguousarray(in_proj_w[N_IN:2 * N_IN].T)
    wv_t = np.ascontiguousarray(in_proj_w[2 * N_IN:3 * N_IN].T)
    wo_t = np.ascontiguousarray(out_proj_w.T)
    bq = in_proj_b[0:N_IN]; bk = in_proj_b[N_IN:2 * N_IN]
    bv = in_proj_b[2 * N_IN:3 * N_IN]
    cwm = conv_w[:, 0].transpose(2, 1, 0)  # [o,ds,n] -> [n,ds,o]

    in_maps, metas = [], []
    for c in range(NCORE):
        b, s0 = c // 4, (c % 4) * TPC
        R = 384 - s0
        x_roll = np.ascontiguousarray(np.roll(x[b], R, axis=0))
        x_roll_t = np.ascontiguousarray(x_roll.T)
        kk = np.arange(S)
        s_k = (kk - R) % S
        s_q = s0 + np.arange(TPC)
        mskv = np.where(s_k[None, :] <= s_q[:, None], 0.0,
                        NEG_INF).astype(np.float32)
        perm = np.r_[np.arange(EPC * c, EPC * (c + 1)),
                     [i for i in range(N_PROC)
                      if not (EPC * c <= i < EPC * (c + 1))]].astype(np.int64)
        dp_c = np.ascontiguousarray(
            down_proj[EPC * c:EPC * (c + 1)].transpose(1, 0, 2).reshape(H, NR2))
        up_c = np.ascontiguousarray(
            up_proj[EPC * c:EPC * (c + 1)].reshape(NR2, H))
        in_maps.append(dict(
            x_roll=x_roll, x_roll_t=x_roll_t, msk=mskv, patterns=patterns, ws_t=ws_t,
            b_shared=b_shared, adl=adl, upa=upa, g1=g1, b1=b1, g2=g2, b2=b2,
            wq_t=wq_t, wk_t=wk_t, wv_t=wv_t, wo_t=wo_t,
            bq=bq, bk=bk, bv=bv, bo=out_proj_b,
            cw=np.ascontiguousarray(cwm[:, :, perm]),
            cb=np.ascontiguousarray(conv_b[perm]),
            dp=dp_c, up=up_c,
        ))
        metas.append(dict(R=R, perm=perm))
    return in_maps, metas


def kernel(**inputs):
    nc = _get_nc()
    in_maps, metas = _prep_inputs(inputs)
    res = run_bass_kernel_spmd(nc, in_maps, list(range(NCORE)))
    rs = res.results

    x2 = np.concatenate([rs[c]["o_x2"] for c in range(NCORE)]).reshape(B, S, H)
    ia = np.concatenate([rs[c]["o_ia"] for c in range(NCORE)]).reshape(B, S, N_IN)
    rel = np.concatenate([rs[c]["o_rel"] for c in range(NCORE)]).reshape(B, S, N_IN)
    enr = np.concatenate([rs[c]["o_enr"] for c in range(NCORE)]).reshape(B, S, N_IN)
    att = np.zeros((B, S, S), np.float32)
    for c in range(NCORE):
        b, s0 = c // 4, (c % 4) * TPC
        att[b, s0:s0 + TPC] = np.roll(rs[c]["o_att"], -metas[c]["R"], axis=1)
    pact = np.zeros((B, S, N_PROC), np.float32)
    pa_all = rs[0]["o_pacts"]  # [N_PROC (perm of core0), B*S]
    inv = np.empty(N_PROC, np.int64)
    inv[metas[0]["perm"]] = np.arange(N_PROC)
    for b in range(B):
        pact[b] = pa_all[:, S * b:S * (b + 1)][inv].T
    return x2, ia, rel, enr, pact, att


# revision 19
# speedup vs baseline: 2.8382x; 2.8382x over previous
"""DAWN layer on 8 trn2 NeuronCores.

Sharding:
- Phase A (input neurons + lateral attention): token-parallel. Core c owns
  128 tokens (batch c//4, s in [(c%4)*128, +128)). Each core receives its
  batch's x ROLLED so its own tokens sit at rows [384:512] (uniform SPMD
  program; causality lives in a per-core additive mask in rolled key order).
- One AllGather moves x1 (pre-transposed to H-major blocks) + enriched_acts
  to every core.
- Phase C (process neurons): expert-parallel. Core c owns experts
  [16c,16c+16) of down_proj/up_proj; conv weights are per-core permuted so
  its experts are rows 0:16 of the conv output. Partial outputs are
  ReduceScattered; each core runs the final LayerNorm on its own tokens.
- Matmuls use float32r (full-rate on TensorE, ~1e-4 rounding).

Host code does data movement only (roll / permute / transpose / reshape).
"""

import contextlib

import numpy as np

import concourse.bass as bass
import concourse.mybir as mybir
import concourse.tile as tile
from concourse import bacc, masks
from concourse.bass_utils import run_bass_kernel_spmd

dt = mybir.dt
AF = mybir.ActivationFunctionType
ALU = mybir.AluOpType
AX = mybir.AxisListType

B, S, H = 2, 512, 768
N_IN, N_PROC, AR, PR = 64, 128, 16, 128
NEG_INF = -1e30
NCORE = 8
TPC = 128               # tokens per core
EPC = N_PROC // NCORE   # experts per core = 16
NR1 = N_IN * AR         # 1024
NR2 = EPC * PR          # 2048
HT = H // 128           # 6
F32 = dt.float32
F32R = dt.float32r
AGW = H + N_IN          # all-gather payload columns (832)


def _f32r(ap):
    return ap.bitcast(F32R)


def build_kernel():
    nc = bacc.Bacc(None)

    def I(name, shape):
        return nc.dram_tensor(name, list(shape), F32, kind="ExternalInput")

    x_roll = I("x_roll", (S, H))
    x_roll_t = I("x_roll_t", (H, S))
    msk = I("msk", (TPC, S))
    patterns = I("patterns", (N_IN, H))
    ws_t = I("ws_t", (H, H))              # w_shared.T -> [h, o]
    b_shared = I("b_shared", (H,))
    adl = I("adl", (H, NR1))              # adapt_down as [h, n*AR+r]
    upa = I("upa", (NR1, H))              # adapt_up as [(n,r), h]
    g1 = I("g1", (H,)); b1 = I("b1", (H,))
    g2 = I("g2", (H,)); b2 = I("b2", (H,))
    wq_t = I("wq_t", (N_IN, N_IN)); wk_t = I("wk_t", (N_IN, N_IN))
    wv_t = I("wv_t", (N_IN, N_IN)); wo_t = I("wo_t", (N_IN, N_IN))
    bq = I("bq", (N_IN,)); bk = I("bk", (N_IN,))
    bv = I("bv", (N_IN,)); bo = I("bo", (N_IN,))
    cw = I("cw", (N_IN, 5, N_PROC))       # conv w [n, ds, o], o permuted per core
    cb = I("cb", (N_PROC,))               # conv bias, permuted per core
    dp = I("dp", (H, NR2))                # down_proj slice [h, e*PR+r]
    up = I("up", (NR2, H))                # up_proj slice [e*PR+r, h]

    def O(name, shape):
        return nc.dram_tensor(name, list(shape), F32, kind="ExternalOutput")

    o_x2 = O("o_x2", (TPC, H))
    o_ia = O("o_ia", (TPC, N_IN))
    o_rel = O("o_rel", (TPC, N_IN))
    o_enr = O("o_enr", (TPC, N_IN))
    o_pacts = O("o_pacts", (N_PROC, B * S))  # both batches, permuted experts
    o_att = O("o_att", (TPC, S))             # rolled key order

    with tile.TileContext(nc) as tc, contextlib.ExitStack() as es:
        dram = es.enter_context(tc.tile_pool(name="dram", bufs=1, space="DRAM"))
        persist = es.enter_context(tc.tile_pool(name="persist", bufs=1))
        bigw = es.enter_context(tc.tile_pool(name="bigw", bufs=1))
        psT = es.enter_context(tc.tile_pool(name="psT", bufs=2, space="PSUM"))

        ag1_in = dram.tile([TPC, N_IN], F32, name="ag1_in", tag="ag1_in")
        ag1_out = dram.tile([NCORE * TPC, N_IN], F32, name="ag1_out",
                            tag="ag1_out", addr_space="Shared")
        ag2_in = dram.tile([TPC, H], F32, name="ag2_in", tag="ag2_in")
        ag2_out = dram.tile([NCORE * TPC, H], F32, name="ag2_out",
                            tag="ag2_out", addr_space="Shared")
        rs_in = dram.tile([NCORE * TPC, H], dt.bfloat16, name="rs_in", tag="rs_in")
        rs_out = dram.tile([TPC, H], dt.bfloat16, name="rs_out", tag="rs_out")
        pacts_d = [dram.tile([N_PROC, S], F32, name=f"pacts_d{b}", tag=f"pacts_d{b}")
                   for b in range(B)]

        # phase-C weights: dedicated space, loads can start immediately
        DP = [bigw.tile([128, NR2], F32R, name=f"dp{i}", tag=f"dp{i}") for i in range(HT)]
        ident = persist.tile([128, 128], F32, name="ident", tag="ident")
        masks.make_identity(nc, ident[:])
        identr = persist.tile([128, 128], F32R, name="identr", tag="identr")
        nc.vector.tensor_copy(identr[:], ident[:])

        def transpose_f32(src_ap):
            """PE-transpose src [p, q] (f32) -> psum tile [q, p]."""
            p, q = src_ap.shape[0], src_ap.shape[-1]
            t = psT.tile([q, p], F32, name="tp", tag="tp")
            nc.tensor.transpose(t[:], src_ap, ident[:p, :p])
            return t

        def brow(pool, name, src, n):
            t = pool.tile([128, n], F32, name=name, tag=name)
            nc.gpsimd.dma_start(out=t, in_=src[:].partition_broadcast(128))
            return t

        BVB = brow(persist, "BVB", bv, N_IN); BOB = brow(persist, "BOB", bo, N_IN)

        def pcol(name, src, n):
            t = persist.tile([n, 1], F32, tag=name)
            nc.sync.dma_start(out=t, in_=src[:].unsqueeze(-1))
            return t

        BQ = pcol("BQ", bq, N_IN); BK = pcol("BK", bk, N_IN)
        CBT = pcol("CBT", cb, N_PROC)

        EPS = persist.tile([128, 1], F32, name="EPS", tag="EPS")
        nc.vector.memset(EPS, 1e-5)
        C8 = persist.tile([128, 1], F32, name="C8", tag="C8")
        nc.vector.memset(C8, 1.0 / float(np.sqrt(N_IN)))

        X1G = persist.tile([TPC, H], F32, name="X1G", tag="X1G")       # own x1

        def layernorm(pool, dst, src, gb, bb):
            st = pool.tile([128, 3, nc.vector.BN_STATS_DIM], F32, name="lnst", tag="lnst")
            s3 = src.rearrange("p (a b) -> p a b", a=3)
            for a in range(3):
                nc.vector.bn_stats(out=st[:, a, :], in_=s3[:, a, :])
            mv = pool.tile([128, nc.vector.BN_AGGR_DIM], F32, name="lnmv", tag="lnmv")
            nc.vector.bn_aggr(out=mv[:], in_=st[:])
            rstd = pool.tile([128, 1], F32, name="lnrstd", tag="lnrstd")
            nc.scalar.activation(rstd[:], mv[:, 1:2], AF.Ln, bias=EPS[:])
            nc.scalar.activation(rstd[:], rstd[:], AF.Exp, scale=-0.5)
            nc.vector.tensor_scalar(out=dst, in0=src, scalar1=mv[:, 0:1],
                                    scalar2=rstd[:], op0=ALU.subtract,
                                    op1=ALU.mult)
            nc.vector.tensor_mul(dst, dst, gb[:])
            nc.vector.tensor_add(dst, dst, bb[:])

        # ================= PHASE A =================
        with contextlib.ExitStack() as esA:
            wsp = esA.enter_context(tc.tile_pool(name="wsp", bufs=2))
            xbp = esA.enter_context(tc.tile_pool(name="xbp", bufs=2))
            xh = esA.enter_context(tc.tile_pool(name="xh", bufs=1))
            sb = esA.enter_context(tc.tile_pool(name="sb", bufs=2))
            att = esA.enter_context(tc.tile_pool(name="att", bufs=1))
            psA = esA.enter_context(tc.tile_pool(name="psA", bufs=6, space="PSUM"))

            BSH = brow(att, "BSH", b_shared, H)
            G1B = brow(att, "G1B", g1, H); B1B = brow(att, "B1B", b1, H)

            # ---- x: H-major direct loads + per-block l2 stats ----
            XH = [xh.tile([128, S], F32R, name=f"XH{i}", tag=f"XH{i}")
                  for i in range(HT)]
            for i in range(HT):
                nc.sync.dma_start(out=XH[i],
                                  in_=_f32r(x_roll_t[128 * i:128 * (i + 1), :]))
            RNt = [att.tile([128, 1], F32, name=f"RN{t}", tag=f"RN{t}")
                   for t in range(4)]
            XB3 = att.tile([128, H], F32, name="XB3", tag="XB3")
            ENRO = att.tile([TPC, N_IN], F32, name="ENRO", tag="ENRO")
            for t in range(4):
                xb = xbp.tile([128, H], F32, name="xb", tag="xb")
                nc.sync.dma_start(out=xb, in_=x_roll[128 * t:128 * (t + 1), :])
                sq = sb.tile([128, H], F32, name="sq", tag="sq")
                nc.vector.tensor_mul(sq[:], xb[:], xb[:])
                ss = sb.tile([128, 1], F32, name="ss", tag="ss")
                nc.vector.reduce_sum(ss[:], sq[:], axis=AX.X)
                nc.scalar.activation(ss[:], ss[:], AF.Ln)
                nc.scalar.activation(RNt[t][:], ss[:], AF.Exp, scale=-0.5)
                if t == 3:
                    nc.vector.tensor_copy(XB3[:], xb[:])

            # ---- patterns: l2-normalize rows, transpose to [h, n] ----
            PT = sb.tile([N_IN, H], F32, name="pt", tag="pt", bufs=1)
            nc.sync.dma_start(out=PT, in_=patterns[:])
            psq = sb.tile([N_IN, H], F32, name="psq", tag="psq", bufs=1)
            nc.vector.tensor_mul(psq[:], PT[:], PT[:])
            prs = sb.tile([N_IN, 1], F32, name="prs", tag="prs")
            nc.vector.reduce_sum(prs[:], psq[:], axis=AX.X)
            nc.scalar.activation(prs[:], prs[:], AF.Ln)
            nc.scalar.activation(prs[:], prs[:], AF.Exp, scale=-0.5)
            nc.vector.tensor_scalar_mul(out=PT[:], in0=PT[:], scalar1=prs[:])
            PNH = [att.tile([128, N_IN], F32R, name=f"PNH{i}", tag=f"PNH{i}")
                   for i in range(HT)]
            for i in range(HT):
                tp = transpose_f32(PT[:, 128 * i:128 * (i + 1)])
                nc.vector.tensor_copy(PNH[i][:], tp[:])

            # ---- input_acts for all 512 tokens ----
            ACTS = att.tile([128, 4 * N_IN], F32, name="ACTS", tag="ACTS")
            for t in range(4):
                rp = psA.tile([128, 512], F32, name="ps", tag="ps")
                for i in range(HT):
                    nc.tensor.matmul(rp[:, :N_IN],
                                     XH[i][:, 128 * t:128 * (t + 1)], PNH[i][:],
                                     start=(i == 0), stop=(i == HT - 1))
                nc.scalar.activation(ACTS[:, N_IN * t:N_IN * (t + 1)],
                                     rp[:, :N_IN], AF.Sigmoid, scale=RNt[t][:])
            AOWN = ACTS[:, N_IN * 3:N_IN * 4]
            nc.sync.dma_start(out=o_ia[:], in_=AOWN)

            ANT = att.tile([N_IN, S], F32R, name="ANT", tag="ANT")
            for t in range(4):
                tp = transpose_f32(ACTS[:, N_IN * t:N_IN * (t + 1)])
                nc.vector.tensor_copy(ANT[:, 128 * t:128 * (t + 1)], tp[:])

            # ---- lateral attention (own 128 queries, all 512 keys) ----
            WQ = att.tile([N_IN, N_IN], F32R, name="WQ", tag="WQ")
            WK = att.tile([N_IN, N_IN], F32R, name="WK", tag="WK")
            WV = att.tile([N_IN, N_IN], F32R, name="WV", tag="WV")
            WO = att.tile([N_IN, N_IN], F32R, name="WO", tag="WO")
            nc.sync.dma_start(out=WQ, in_=_f32r(wq_t[:]))
            nc.sync.dma_start(out=WK, in_=_f32r(wk_t[:]))
            nc.sync.dma_start(out=WV, in_=_f32r(wv_t[:]))
            nc.sync.dma_start(out=WO, in_=_f32r(wo_t[:]))
            MT = att.tile([128, S], F32, name="MT", tag="MT")
            nc.sync.dma_start(out=MT, in_=msk[:])

            qp = psA.tile([128, 512], F32, name="ps", tag="ps")
            nc.tensor.matmul(qp[:N_IN, :TPC], WQ[:], ANT[:, 384:512],
                             start=True, stop=True)
            QM = att.tile([N_IN, TPC], F32R, name="QM", tag="QM")
            nc.vector.tensor_scalar(out=QM[:], in0=qp[:N_IN, :TPC],
                                    scalar1=BQ[:], scalar2=None, op0=ALU.add)
            kp = psA.tile([128, 512], F32, name="ps", tag="ps")
            nc.tensor.matmul(kp[:N_IN, :], WK[:], ANT[:], start=True, stop=True)
            KM = att.tile([N_IN, S], F32R, name="KM", tag="KM")
            nc.vector.tensor_scalar(out=KM[:], in0=kp[:N_IN, :],
                                    scalar1=BK[:], scalar2=None, op0=ALU.add)
            VT = att.tile([128, 4 * N_IN], F32R, name="VT", tag="VT")
            for t in range(4):
                vp = psA.tile([128, 512], F32, name="ps", tag="ps")
                nc.tensor.matmul(vp[:, :N_IN], ANT[:, 128 * t:128 * (t + 1)],
                                 WV[:], start=True, stop=True)
                nc.vector.tensor_add(VT[:, N_IN * t:N_IN * (t + 1)],
                                     vp[:, :N_IN], BVB[:])
            scp = psA.tile([128, 512], F32, name="ps", tag="ps")
            nc.tensor.matmul(scp[:], QM[:], KM[:], start=True, stop=True)
            SCM = att.tile([128, S], F32, name="SCM", tag="SCM")
            nc.vector.tensor_scalar_mul(out=SCM[:], in0=scp[:], scalar1=C8[:])
            nc.vector.tensor_add(SCM[:], SCM[:], MT[:])
            NM = sb.tile([128, 1], F32, name="NM", tag="NM")
            nc.vector.reduce_max(NM[:], SCM[:], axis=AX.X, negate=True)
            ATT = att.tile([128, S], F32, name="ATT", tag="ATT")
            nc.scalar.activation(ATT[:], SCM[:], AF.Exp, bias=NM[:])
            DN = sb.tile([128, 1], F32, name="DN", tag="DN")
            nc.vector.reduce_sum(DN[:], ATT[:], axis=AX.X)
            nc.vector.reciprocal(DN[:], DN[:])
            nc.vector.tensor_scalar_mul(out=ATT[:], in0=ATT[:], scalar1=DN[:])
            nc.sync.dma_start(out=o_att[:], in_=ATT[:])
            ATK = [att.tile([128, 128], F32R, name=f"ATK{t}", tag=f"ATK{t}")
                   for t in range(4)]
            for t in range(4):
                tp = transpose_f32(ATT[:, 128 * t:128 * (t + 1)])
                nc.vector.tensor_copy(ATK[t][:], tp[:])
            ctxp = psA.tile([128, 512], F32, name="ps", tag="ps")
            for t in range(4):
                nc.tensor.matmul(ctxp[:, :N_IN], ATK[t][:],
                                 VT[:, N_IN * t:N_IN * (t + 1)],
                                 start=(t == 0), stop=(t == 3))
            CTX = sb.tile([128, N_IN], F32, name="CTX", tag="CTX")
            nc.vector.tensor_copy(CTX[:], ctxp[:, :N_IN])
            tp = transpose_f32(CTX[:])
            CTN = sb.tile([N_IN, TPC], F32R, name="CTN", tag="CTN")
            nc.vector.tensor_copy(CTN[:], tp[:])
            relp = psA.tile([128, 512], F32, name="ps", tag="ps")
            nc.tensor.matmul(relp[:, :N_IN], CTN[:], WO[:], start=True, stop=True)
            RELS = sb.tile([TPC, N_IN], F32, name="RELS", tag="RELS")
            nc.vector.tensor_add(RELS[:], relp[:, :N_IN], BOB[:])
            nc.sync.dma_start(out=o_rel[:], in_=RELS[:])
            nc.vector.tensor_add(ENRO[:], RELS[:], AOWN)
            nc.sync.dma_start(out=o_enr[:], in_=ENRO[:])
            nc.sync.dma_start(out=ag1_in[:], in_=ENRO[:])

            # enriched all-gather fires while the shared/adapter path runs
            nc.gpsimd.collective_compute(
                "AllGather", ALU.bypass,
                replica_groups=[list(range(NCORE))],
                ins=[ag1_in[:].opt()], outs=[ag1_out[:].opt()],
            )

            # ---- shared = gelu(x @ Wsh + b) (own tokens) ----
            SHP = [psA.tile([128, 512], F32, name="ps", tag="ps") for _ in range(2)]
            for i in range(HT):
                wst = wsp.tile([128, H], F32R, name="ws", tag="ws")
                nc.scalar.dma_start(out=wst,
                                    in_=_f32r(ws_t[128 * i:128 * (i + 1), :]))
                for h2 in range(2):
                    nc.tensor.matmul(SHP[h2][:, :384], XH[i][:, 384:512],
                                     wst[:, 384 * h2:384 * (h2 + 1)],
                                     start=(i == 0), stop=(i == HT - 1))
            SHG = sb.tile([128, H], F32, name="SHG", tag="SHG", bufs=1)
            for h2 in range(2):
                nc.vector.tensor_add(SHG[:, 384 * h2:384 * (h2 + 1)],
                                     SHP[h2][:, :384],
                                     BSH[:, 384 * h2:384 * (h2 + 1)])
            nc.scalar.activation(SHG[:], SHG[:], AF.Gelu)
            SHH = [att.tile([128, 128], F32R, name=f"SHH{i}", tag=f"SHH{i}")
                   for i in range(HT)]
            for i in range(HT):
                tp = transpose_f32(SHG[:, 128 * i:128 * (i + 1)])
                nc.vector.tensor_copy(SHH[i][:], tp[:])

            # ---- adapters: down -> scale by acts -> spec ----
            SDT = sb.tile([128, NR1], F32R, name="SDT", tag="SDT", bufs=1)
            ADT = [wsp.tile([128, NR1], F32R, name=f"adl{i}", tag=f"adl{i}",
                            bufs=1) for i in range(HT)]
            for i in range(HT):
                nc.scalar.dma_start(out=ADT[i],
                                    in_=_f32r(adl[128 * i:128 * (i + 1), :]))
            for nh in range(2):
                dpp = psA.tile([128, 512], F32, name="ps", tag="ps")
                for i in range(HT):
                    nc.tensor.matmul(dpp[:], SHH[i][:],
                                     ADT[i][:, 512 * nh:512 * (nh + 1)],
                                     start=(i == 0), stop=(i == HT - 1))
                ab = ACTS[:, N_IN * 3 + 32 * nh:N_IN * 3 + 32 * (nh + 1)]
                abc = bass.AP(tensor=ab.tensor, offset=ab.offset,
                              ap=[list(ab.ap[0]), [ab.ap[1][0], 32], [0, AR]])
                sdt3 = SDT[:, 512 * nh:512 * (nh + 1)].rearrange(
                    "p (a b) -> p a b", a=32)
                dpp3 = dpp[:].rearrange("p (a b) -> p a b", a=32)
                nc.vector.tensor_tensor(out=sdt3, in0=dpp3, in1=abc, op=ALU.mult)
            SDN = [att.tile([128, 128], F32R, name=f"SDN{j}", tag=f"SDN{j}")
                   for j in range(8)]
            for j in range(8):
                t = psT.tile([128, 128], F32, name="tp", tag="tp")
                nc.tensor.transpose(t[:].bitcast(F32R),
                                    SDT[:, 128 * j:128 * (j + 1)], identr[:])
                nc.vector.tensor_copy(SDN[j][:], t[:].bitcast(F32R))
            SPP = [psA.tile([128, 512], F32, name="ps", tag="ps") for _ in range(2)]
            for j in range(8):
                upt = wsp.tile([128, H], F32R, name="upa", tag="upa")
                nc.scalar.dma_start(out=upt,
                                    in_=_f32r(upa[128 * j:128 * (j + 1), :]))
                for h2 in range(2):
                    nc.tensor.matmul(SPP[h2][:, :384], SDN[j][:],
                                     upt[:, 384 * h2:384 * (h2 + 1)],
                                     start=(j == 0), stop=(j == 7))

            # ---- intermediate + LN1 -> x1 ----
            SA = sb.tile([128, 1], F32, name="SA", tag="SA")
            nc.vector.reduce_sum(SA[:], AOWN, axis=AX.X)
            XI = sb.tile([128, H], F32, name="XI", tag="XI", bufs=1)
            nc.vector.tensor_scalar_mul(out=XI[:], in0=SHG[:], scalar1=SA[:])
            for h2 in range(2):
                nc.vector.tensor_add(XI[:, 384 * h2:384 * (h2 + 1)],
                                     XI[:, 384 * h2:384 * (h2 + 1)],
                                     SPP[h2][:, :384])
            nc.vector.tensor_add(XI[:], XI[:], XB3[:])
            layernorm(sb, X1G[:], XI[:], G1B, B1B)

            # pack x1 H-major into ag2_in
            for i in range(HT):
                tp = transpose_f32(X1G[:, 128 * i:128 * (i + 1)])
                xhc = sb.tile([128, 128], F32, name="xhc", tag="xhc")
                nc.vector.tensor_copy(xhc[:], tp[:])
                nc.sync.dma_start(out=ag2_in[:, 128 * i:128 * (i + 1)], in_=xhc[:])

        # ================= ALLGATHER (x1) =================
        nc.gpsimd.collective_compute(
            "AllGather", ALU.bypass,
            replica_groups=[list(range(NCORE))],
            ins=[ag2_in[:].opt()], outs=[ag2_out[:].opt()],
        )

        # ================= PHASE C =================
        with contextlib.ExitStack() as esC:
            cp = esC.enter_context(tc.tile_pool(name="cp", bufs=2))
            x1p = esC.enter_context(tc.tile_pool(name="x1p", bufs=1))
            sdp = esC.enter_context(tc.tile_pool(name="sdp", bufs=1))
            upp = esC.enter_context(tc.tile_pool(name="upp", bufs=1))
            bc = esC.enter_context(tc.tile_pool(name="bc", bufs=2))
            psC = esC.enter_context(tc.tile_pool(name="psC", bufs=2, space="PSUM"))
            psY = esC.enter_context(tc.tile_pool(name="psY", bufs=4, space="PSUM"))

            # big weights now; DP tiles were reserved up-front
            for i in range(HT):
                nc.gpsimd.dma_start(out=DP[i],
                                    in_=_f32r(dp[128 * i:128 * (i + 1), :]))
            UP = [upp.tile([128, H], F32R, name=f"up{e}", tag=f"up{e}")
                  for e in range(EPC)]
            for e in range(EPC):
                nc.gpsimd.dma_start(out=UP[e],
                                    in_=_f32r(up[128 * e:128 * (e + 1), :]))

            PACTS = [x1p.tile([N_PROC, S], F32, name=f"PACTS{b}", tag=f"PACTS{b}")
                     for b in range(B)]
            G2B = brow(x1p, "G2B", g2, H); B2B = brow(x1p, "B2B", b2, H)

            # conv over gathered enriched -> process_acts^T per batch
            CWT = x1p.tile([N_IN, 5 * N_PROC], F32R, name="CWT", tag="CWT")
            nc.sync.dma_start(out=CWT,
                              in_=_f32r(cw[:].rearrange("n a o -> n (a o)")))
            ENT = [x1p.tile([N_IN, S + 4], F32R, name=f"ENT{b}", tag=f"ENT{b}")
                   for b in range(B)]
            ZC = x1p.tile([N_IN, 2], F32, name="ZC", tag="ZC")
            nc.vector.memset(ZC, 0.0)
            for b in range(B):
                nc.vector.tensor_copy(ENT[b][:, 0:2], ZC[:])
                nc.vector.tensor_copy(ENT[b][:, S + 2:S + 4], ZC[:])
            for c in range(NCORE):
                b, blk = c // 4, c % 4
                ec = cp.tile([TPC, N_IN], F32, name="ec", tag="ec")
                nc.sync.dma_start(out=ec, in_=ag1_out[TPC * c:TPC * (c + 1), :])
                tp = transpose_f32(ec[:])
                nc.vector.tensor_copy(
                    ENT[b][:, 2 + 128 * blk:2 + 128 * (blk + 1)], tp[:])
            for b in range(B):
                rsp = psC.tile([N_PROC, S], F32, name="pd", tag="pd")
                for ds in range(5):
                    nc.tensor.matmul(rsp[:], CWT[:, N_PROC * ds:N_PROC * (ds + 1)],
                                     ENT[b][:, ds:ds + S],
                                     start=(ds == 0), stop=(ds == 4))
                nc.scalar.activation(PACTS[b][:], rsp[:], AF.Sigmoid, bias=CBT[:])
                nc.sync.dma_start(out=o_pacts[:, S * b:S * (b + 1)],
                                  in_=PACTS[b][:])
                nc.sync.dma_start(out=pacts_d[b][:], in_=PACTS[b][:])

            # PE warm-keeper: chained f32 matmuls on x1 bridging the AG2 wait
            wup = psC.tile([128, 512], F32, name="wup", tag="pd")
            for w in range(8):
                nc.tensor.matmul(wup[:], X1G[:, 0:128], X1G[:, 0:512],
                                 start=(w == 0), stop=(w == 7))

            SD = [sdp.tile([128, S], F32R, name=f"SD{e}", tag=f"SD{e}")
                  for e in range(EPC)]
            for b in range(B):
                # x1 H-major (this batch) from the gathered payload
                X1H = [x1p.tile([128, S], F32R, name=f"X1H{i}", tag=f"X1H{i}")
                       for i in range(HT)]
                for i in range(HT):
                    for cc in range(4):
                        c = 4 * b + cc
                        nc.sync.dma_start(
                            out=X1H[i][:, TPC * cc:TPC * (cc + 1)],
                            in_=_f32r(ag2_out[TPC * c:TPC * (c + 1),
                                              128 * i:128 * (i + 1)]))
                # pass 1: down-proj + gate by process_acts
                for e in range(EPC):
                    pd = psC.tile([PR, S], F32, name="pd", tag="pd")
                    for i in range(HT):
                        nc.tensor.matmul(pd[:], DP[i][:, PR * e:PR * (e + 1)],
                                         X1H[i][:],
                                         start=(i == 0), stop=(i == HT - 1))
                    bce = bc.tile([PR, S], F32, name="bce", tag="bce")
                    nc.gpsimd.dma_start(
                        out=bce, in_=pacts_d[b][e].partition_broadcast(PR))
                    nc.vector.tensor_tensor(out=SD[e][:], in0=pd[:], in1=bce[:],
                                            op=ALU.mult)
                # pass 2: up-proj, accumulate over experts in PSUM
                for tb in range(4):
                    yp = [psY.tile([128, 384], F32, name="yp", tag="yp")
                          for _ in range(2)]
                    for e in range(EPC):
                        for h2 in range(2):
                            nc.tensor.matmul(yp[h2][:],
                                             SD[e][:, 128 * tb:128 * (tb + 1)],
                                             UP[e][:, 384 * h2:384 * (h2 + 1)],
                                             start=(e == 0), stop=(e == EPC - 1))
                    ys = cp.tile([128, H], dt.bfloat16, name="ys", tag="ys")
                    for h2 in range(2):
                        nc.vector.tensor_copy(ys[:, 384 * h2:384 * (h2 + 1)],
                                              yp[h2][:])
                    nc.sync.dma_start(
                        out=rs_in[S * b + 128 * tb:S * b + 128 * (tb + 1), :],
                        in_=ys[:])

            nc.gpsimd.collective_compute(
                "ReduceScatter", ALU.add,
                replica_groups=[list(range(NCORE))],
                ins=[rs_in[:].opt()], outs=[rs_out[:].opt()],
            )

            YRb = cp.tile([TPC, H], dt.bfloat16, name="YRb", tag="YRb", bufs=1)
            nc.sync.dma_start(out=YRb, in_=rs_out[:])
            YR = cp.tile([TPC, H], F32, name="YR", tag="YR", bufs=1)
            nc.vector.tensor_add(YR[:], YRb[:], X1G[:])
            layernorm(cp, YR[:], YR[:], G2B, B2B)
            nc.sync.dma_start(out=o_x2[:], in_=YR[:])

    nc.finalize()
    return nc


_NC_CACHE = None


def _get_nc():
    global _NC_CACHE
    if _NC_CACHE is None:
        _NC_CACHE = build_kernel()
    return _NC_CACHE


def _prep_inputs(inputs):
    f = lambda k: np.asarray(inputs[k], dtype=np.float32)
    x = f("x"); patterns = f("patterns")
    w_shared = f("w_shared"); b_shared = f("b_shared")
    adapt_down = f("adapt_down"); adapt_up = f("adapt_up")
    g1 = f("g1"); b1 = f("b1"); g2 = f("g2"); b2 = f("b2")
    in_proj_w = f("in_proj_w"); in_proj_b = f("in_proj_b")
    out_proj_w = f("out_proj_w"); out_proj_b = f("out_proj_b")
    conv_w = f("conv_w"); conv_b = f("conv_b")
    down_proj = f("down_proj"); up_proj = f("up_proj")

    adl = np.ascontiguousarray(adapt_down.transpose(1, 0, 2).reshape(H, NR1))
    upa = np.ascontiguousarray(adapt_up.reshape(NR1, H))
    ws_t = np.ascontiguousarray(w_shared.T)  # 'bsh,oh->bso': rows of w are o
    wq_t = np.ascontiguousarray(in_proj_w[0:N_IN].T)
    wk_t = np.ascontiguousarray(in_proj_w[N_IN:2 * N_IN].T)
    wv_t = np.ascontiguousarray(in_proj_w[2 * N_IN:3 * N_IN].T)
    wo_t = np.ascontiguousarray(out_proj_w.T)
    bq = in_proj_b[0:N_IN]; bk = in_proj_b[N_IN:2 * N_IN]
    bv = in_proj_b[2 * N_IN:3 * N_IN]
    cwm = conv_w[:, 0].transpose(2, 1, 0)  # [o,ds,n] -> [n,ds,o]

    in_maps, metas = [], []
    for c in range(NCORE):
        b, s0 = c // 4, (c % 4) * TPC
        R = 384 - s0
        x_roll = np.ascontiguousarray(np.roll(x[b], R, axis=0))
        x_roll_t = np.ascontiguousarray(x_roll.T)
        kk = np.arange(S)
        s_k = (kk - R) % S
        s_q = s0 + np.arange(TPC)
        mskv = np.where(s_k[None, :] <= s_q[:, None], 0.0,
                        NEG_INF).astype(np.float32)
        perm = np.r_[np.arange(EPC * c, EPC * (c + 1)),
                     [i for i in range(N_PROC)
                      if not (EPC * c <= i < EPC * (c + 1))]].astype(np.int64)
        dp_c = np.ascontiguousarray(
            down_proj[EPC * c:EPC * (c + 1)].transpose(1, 0, 2).reshape(H, NR2))
        up_c = np.ascontiguousarray(
            up_proj[EPC * c:EPC * (c + 1)].reshape(NR2, H))
        in_maps.append(dict(
            x_roll=x_roll, x_roll_t=x_roll_t, msk=mskv, patterns=patterns, ws_t=ws_t,
            b_shared=b_shared, adl=adl, upa=upa, g1=g1, b1=b1, g2=g2, b2=b2,
            wq_t=wq_t, wk_t=wk_t, wv_t=wv_t, wo_t=wo_t,
            bq=bq, bk=bk, bv=bv, bo=out_proj_b,
            cw=np.ascontiguousarray(cwm[:, :, perm]),
            cb=np.ascontiguousarray(conv_b[perm]),
            dp=dp_c, up=up_c,
        ))
        metas.append(dict(R=R, perm=perm))
    return in_maps, metas


def kernel(**inputs):
    nc = _get_nc()
    in_maps, metas = _prep_inputs(inputs)
    res = run_bass_kernel_spmd(nc, in_maps, list(range(NCORE)))
    rs = res.results

    x2 = np.concatenate([rs[c]["o_x2"] for c in range(NCORE)]).reshape(B, S, H)
    ia = np.concatenate([rs[c]["o_ia"] for c in range(NCORE)]).reshape(B, S, N_IN)
    rel = np.concatenate([rs[c]["o_rel"] for c in range(NCORE)]).reshape(B, S, N_IN)
    enr = np.concatenate([rs[c]["o_enr"] for c in range(NCORE)]).reshape(B, S, N_IN)
    att = np.zeros((B, S, S), np.float32)
    for c in range(NCORE):
        b, s0 = c // 4, (c % 4) * TPC
        att[b, s0:s0 + TPC] = np.roll(rs[c]["o_att"], -metas[c]["R"], axis=1)
    pact = np.zeros((B, S, N_PROC), np.float32)
    pa_all = rs[0]["o_pacts"]  # [N_PROC (perm of core0), B*S]
    inv = np.empty(N_PROC, np.int64)
    inv[metas[0]["perm"]] = np.arange(N_PROC)
    for b in range(B):
        pact[b] = pa_all[:, S * b:S * (b + 1)][inv].T
    return x2, ia, rel, enr, pact, att


# revision 20
# speedup vs baseline: 3.0249x; 1.0658x over previous
"""DAWN layer on 8 trn2 NeuronCores.

Sharding:
- Phase A (input neurons + lateral attention): token-parallel. Core c owns
  128 tokens (batch c//4, s in [(c%4)*128, +128)). Each core receives its
  batch's x ROLLED so its own tokens sit at rows [384:512] (uniform SPMD
  program; causality lives in a per-core additive mask in rolled key order).
- One AllGather moves x1 (pre-transposed to H-major blocks) + enriched_acts
  to every core.
- Phase C (process neurons): expert-parallel. Core c owns experts
  [16c,16c+16) of down_proj/up_proj; conv weights are per-core permuted so
  its experts are rows 0:16 of the conv output. Partial outputs are
  ReduceScattered; each core runs the final LayerNorm on its own tokens.
- Matmuls use float32r (full-rate on TensorE, ~1e-4 rounding).

Host code does data movement only (roll / permute / transpose / reshape).
"""

import contextlib

import numpy as np

import concourse.bass as bass
import concourse.mybir as mybir
import concourse.tile as tile
from concourse import bacc, masks
from concourse.bass_utils import run_bass_kernel_spmd

dt = mybir.dt
AF = mybir.ActivationFunctionType
ALU = mybir.AluOpType
AX = mybir.AxisListType

B, S, H = 2, 512, 768
N_IN, N_PROC, AR, PR = 64, 128, 16, 128
NEG_INF = -1e30
NCORE = 8
TPC = 128               # tokens per core
EPC = N_PROC // NCORE   # experts per core = 16
NR1 = N_IN * AR         # 1024
NR2 = EPC * PR          # 2048
HT = H // 128           # 6
F32 = dt.float32
F32R = dt.float32r
AGW = H + N_IN          # all-gather payload columns (832)


def _f32r(ap):
    return ap.bitcast(F32R)


def build_kernel():
    nc = bacc.Bacc(None)

    def I(name, shape):
        return nc.dram_tensor(name, list(shape), F32, kind="ExternalInput")

    x_roll = I("x_roll", (S, H))
    x_roll_t = I("x_roll_t", (H, S))
    msk = I("msk", (TPC, S))
    patterns = I("patterns", (N_IN, H))
    ws_t = I("ws_t", (H, H))              # w_shared.T -> [h, o]
    b_shared = I("b_shared", (H,))
    adl = I("adl", (H, NR1))              # adapt_down as [h, n*AR+r]
    upa = I("upa", (NR1, H))              # adapt_up as [(n,r), h]
    g1 = I("g1", (H,)); b1 = I("b1", (H,))
    g2 = I("g2", (H,)); b2 = I("b2", (H,))
    wq_t = I("wq_t", (N_IN, N_IN)); wk_t = I("wk_t", (N_IN, N_IN))
    wv_t = I("wv_t", (N_IN, N_IN)); wo_t = I("wo_t", (N_IN, N_IN))
    bq = I("bq", (N_IN,)); bk = I("bk", (N_IN,))
    bv = I("bv", (N_IN,)); bo = I("bo", (N_IN,))
    cw = I("cw", (N_IN, 5, N_PROC))       # conv w [n, ds, o], o permuted per core
    cb = I("cb", (N_PROC,))               # conv bias, permuted per core
    dp = I("dp", (H, NR2))                # down_proj slice [h, e*PR+r]
    up = I("up", (NR2, H))                # up_proj slice [e*PR+r, h]

    def O(name, shape):
        return nc.dram_tensor(name, list(shape), F32, kind="ExternalOutput")

    o_x2 = O("o_x2", (TPC, H))
    o_ia = O("o_ia", (TPC, N_IN))
    o_rel = O("o_rel", (TPC, N_IN))
    o_enr = O("o_enr", (TPC, N_IN))
    o_pacts = O("o_pacts", (N_PROC, B * S))  # both batches, permuted experts
    o_att = O("o_att", (TPC, S))             # rolled key order

    with tile.TileContext(nc) as tc, contextlib.ExitStack() as es:
        dram = es.enter_context(tc.tile_pool(name="dram", bufs=1, space="DRAM"))
        persist = es.enter_context(tc.tile_pool(name="persist", bufs=1))
        bigw = es.enter_context(tc.tile_pool(name="bigw", bufs=1))
        psT = es.enter_context(tc.tile_pool(name="psT", bufs=2, space="PSUM"))

        ag1_in = dram.tile([TPC, N_IN], F32, name="ag1_in", tag="ag1_in")
        ag1_out = dram.tile([NCORE * TPC, N_IN], F32, name="ag1_out",
                            tag="ag1_out", addr_space="Shared")
        ag2_in = dram.tile([TPC, H], F32, name="ag2_in", tag="ag2_in")
        ag2_out = dram.tile([NCORE * TPC, H], F32, name="ag2_out",
                            tag="ag2_out", addr_space="Shared")
        rs_in = dram.tile([NCORE * TPC, H], dt.bfloat16, name="rs_in", tag="rs_in")
        rs_out = dram.tile([TPC, H], dt.bfloat16, name="rs_out", tag="rs_out")
        pacts_d = [dram.tile([N_PROC, S], F32, name=f"pacts_d{b}", tag=f"pacts_d{b}")
                   for b in range(B)]

        # phase-C weights: dedicated space, loads can start immediately
        DP = [bigw.tile([128, NR2], F32R, name=f"dp{i}", tag=f"dp{i}") for i in range(HT)]
        ident = persist.tile([128, 128], F32, name="ident", tag="ident")
        masks.make_identity(nc, ident[:])
        identr = persist.tile([128, 128], F32R, name="identr", tag="identr")
        nc.vector.tensor_copy(identr[:], ident[:])

        def transpose_f32(src_ap):
            """PE-transpose src [p, q] (f32) -> psum tile [q, p]."""
            p, q = src_ap.shape[0], src_ap.shape[-1]
            t = psT.tile([q, p], F32, name="tp", tag="tp")
            nc.tensor.transpose(t[:], src_ap, ident[:p, :p])
            return t

        def brow(pool, name, src, n):
            t = pool.tile([128, n], F32, name=name, tag=name)
            nc.gpsimd.dma_start(out=t, in_=src[:].partition_broadcast(128))
            return t

        BVB = brow(persist, "BVB", bv, N_IN); BOB = brow(persist, "BOB", bo, N_IN)

        def pcol(name, src, n):
            t = persist.tile([n, 1], F32, tag=name)
            nc.sync.dma_start(out=t, in_=src[:].unsqueeze(-1))
            return t

        BQ = pcol("BQ", bq, N_IN); BK = pcol("BK", bk, N_IN)
        CBT = pcol("CBT", cb, N_PROC)

        EPS = persist.tile([128, 1], F32, name="EPS", tag="EPS")
        nc.vector.memset(EPS, 1e-5)
        C8 = persist.tile([128, 1], F32, name="C8", tag="C8")
        nc.vector.memset(C8, 1.0 / float(np.sqrt(N_IN)))

        X1G = persist.tile([TPC, H], F32, name="X1G", tag="X1G")       # own x1

        def layernorm(pool, dst, src, gb, bb):
            st = pool.tile([128, 3, nc.vector.BN_STATS_DIM], F32, name="lnst", tag="lnst")
            s3 = src.rearrange("p (a b) -> p a b", a=3)
            for a in range(3):
                nc.vector.bn_stats(out=st[:, a, :], in_=s3[:, a, :])
            mv = pool.tile([128, nc.vector.BN_AGGR_DIM], F32, name="lnmv", tag="lnmv")
            nc.vector.bn_aggr(out=mv[:], in_=st[:])
            rstd = pool.tile([128, 1], F32, name="lnrstd", tag="lnrstd")
            nc.scalar.activation(rstd[:], mv[:, 1:2], AF.Ln, bias=EPS[:])
            nc.scalar.activation(rstd[:], rstd[:], AF.Exp, scale=-0.5)
            nc.vector.tensor_scalar(out=dst, in0=src, scalar1=mv[:, 0:1],
                                    scalar2=rstd[:], op0=ALU.subtract,
                                    op1=ALU.mult)
            nc.vector.tensor_mul(dst, dst, gb[:])
            nc.vector.tensor_add(dst, dst, bb[:])

        # ================= PHASE A =================
        with contextlib.ExitStack() as esA:
            wsp = esA.enter_context(tc.tile_pool(name="wsp", bufs=2))
            xbp = esA.enter_context(tc.tile_pool(name="xbp", bufs=2))
            xh = esA.enter_context(tc.tile_pool(name="xh", bufs=1))
            sb = esA.enter_context(tc.tile_pool(name="sb", bufs=2))
            att = esA.enter_context(tc.tile_pool(name="att", bufs=1))
            psA = esA.enter_context(tc.tile_pool(name="psA", bufs=6, space="PSUM"))

            BSH = brow(att, "BSH", b_shared, H)
            G1B = brow(att, "G1B", g1, H); B1B = brow(att, "B1B", b1, H)

            # ---- x: H-major direct loads + per-block l2 stats ----
            XH = [xh.tile([128, S], F32R, name=f"XH{i}", tag=f"XH{i}")
                  for i in range(HT)]
            for i in range(HT):
                nc.sync.dma_start(out=XH[i],
                                  in_=_f32r(x_roll_t[128 * i:128 * (i + 1), :]))
            RNt = [att.tile([128, 1], F32, name=f"RN{t}", tag=f"RN{t}")
                   for t in range(4)]
            XB3 = att.tile([128, H], F32, name="XB3", tag="XB3")
            ENRO = att.tile([TPC, N_IN], F32, name="ENRO", tag="ENRO")
            for t in range(4):
                xb = xbp.tile([128, H], F32, name="xb", tag="xb")
                nc.sync.dma_start(out=xb, in_=x_roll[128 * t:128 * (t + 1), :])
                sq = sb.tile([128, H], F32, name="sq", tag="sq")
                nc.vector.tensor_mul(sq[:], xb[:], xb[:])
                ss = sb.tile([128, 1], F32, name="ss", tag="ss")
                nc.vector.reduce_sum(ss[:], sq[:], axis=AX.X)
                nc.scalar.activation(ss[:], ss[:], AF.Ln)
                nc.scalar.activation(RNt[t][:], ss[:], AF.Exp, scale=-0.5)
                if t == 3:
                    nc.vector.tensor_copy(XB3[:], xb[:])

            # ---- patterns: l2-normalize rows, transpose to [h, n] ----
            PT = sb.tile([N_IN, H], F32, name="pt", tag="pt", bufs=1)
            nc.sync.dma_start(out=PT, in_=patterns[:])
            psq = sb.tile([N_IN, H], F32, name="psq", tag="psq", bufs=1)
            nc.vector.tensor_mul(psq[:], PT[:], PT[:])
            prs = sb.tile([N_IN, 1], F32, name="prs", tag="prs")
            nc.vector.reduce_sum(prs[:], psq[:], axis=AX.X)
            nc.scalar.activation(prs[:], prs[:], AF.Ln)
            nc.scalar.activation(prs[:], prs[:], AF.Exp, scale=-0.5)
            nc.vector.tensor_scalar_mul(out=PT[:], in0=PT[:], scalar1=prs[:])
            PNH = [att.tile([128, N_IN], F32R, name=f"PNH{i}", tag=f"PNH{i}")
                   for i in range(HT)]
            for i in range(HT):
                tp = transpose_f32(PT[:, 128 * i:128 * (i + 1)])
                nc.vector.tensor_copy(PNH[i][:], tp[:])

            # ---- input_acts for all 512 tokens ----
            ACTS = att.tile([128, 4 * N_IN], F32, name="ACTS", tag="ACTS")
            for t in range(4):
                rp = psA.tile([128, 512], F32, name="ps", tag="ps")
                for i in range(HT):
                    nc.tensor.matmul(rp[:, :N_IN],
                                     XH[i][:, 128 * t:128 * (t + 1)], PNH[i][:],
                                     start=(i == 0), stop=(i == HT - 1))
                nc.scalar.activation(ACTS[:, N_IN * t:N_IN * (t + 1)],
                                     rp[:, :N_IN], AF.Sigmoid, scale=RNt[t][:])
            AOWN = ACTS[:, N_IN * 3:N_IN * 4]
            nc.sync.dma_start(out=o_ia[:], in_=AOWN)

            ANT = att.tile([N_IN, S], F32R, name="ANT", tag="ANT")
            for t in range(4):
                tp = transpose_f32(ACTS[:, N_IN * t:N_IN * (t + 1)])
                nc.vector.tensor_copy(ANT[:, 128 * t:128 * (t + 1)], tp[:])

            # ---- lateral attention (own 128 queries, all 512 keys) ----
            WQ = att.tile([N_IN, N_IN], F32R, name="WQ", tag="WQ")
            WK = att.tile([N_IN, N_IN], F32R, name="WK", tag="WK")
            WV = att.tile([N_IN, N_IN], F32R, name="WV", tag="WV")
            WO = att.tile([N_IN, N_IN], F32R, name="WO", tag="WO")
            nc.sync.dma_start(out=WQ, in_=_f32r(wq_t[:]))
            nc.sync.dma_start(out=WK, in_=_f32r(wk_t[:]))
            nc.sync.dma_start(out=WV, in_=_f32r(wv_t[:]))
            nc.sync.dma_start(out=WO, in_=_f32r(wo_t[:]))
            MT = att.tile([128, S], F32, name="MT", tag="MT")
            nc.sync.dma_start(out=MT, in_=msk[:])

            qp = psA.tile([128, 512], F32, name="ps", tag="ps")
            nc.tensor.matmul(qp[:N_IN, :TPC], WQ[:], ANT[:, 384:512],
                             start=True, stop=True)
            QM = att.tile([N_IN, TPC], F32R, name="QM", tag="QM")
            nc.vector.tensor_scalar(out=QM[:], in0=qp[:N_IN, :TPC],
                                    scalar1=BQ[:], scalar2=None, op0=ALU.add)
            kp = psA.tile([128, 512], F32, name="ps", tag="ps")
            nc.tensor.matmul(kp[:N_IN, :], WK[:], ANT[:], start=True, stop=True)
            KM = att.tile([N_IN, S], F32R, name="KM", tag="KM")
            nc.vector.tensor_scalar(out=KM[:], in0=kp[:N_IN, :],
                                    scalar1=BK[:], scalar2=None, op0=ALU.add)
            VT = att.tile([128, 4 * N_IN], F32R, name="VT", tag="VT")
            for t in range(4):
                vp = psA.tile([128, 512], F32, name="ps", tag="ps")
                nc.tensor.matmul(vp[:, :N_IN], ANT[:, 128 * t:128 * (t + 1)],
                                 WV[:], start=True, stop=True)
                nc.vector.tensor_add(VT[:, N_IN * t:N_IN * (t + 1)],
                                     vp[:, :N_IN], BVB[:])
            scp = psA.tile([128, 512], F32, name="ps", tag="ps")
            nc.tensor.matmul(scp[:], QM[:], KM[:], start=True, stop=True)
            SCM = att.tile([128, S], F32, name="SCM", tag="SCM")
            nc.vector.tensor_scalar_mul(out=SCM[:], in0=scp[:], scalar1=C8[:])
            nc.vector.tensor_add(SCM[:], SCM[:], MT[:])
            NM = sb.tile([128, 1], F32, name="NM", tag="NM")
            nc.vector.reduce_max(NM[:], SCM[:], axis=AX.X, negate=True)
            ATT = att.tile([128, S], F32, name="ATT", tag="ATT")
            nc.scalar.activation(ATT[:], SCM[:], AF.Exp, bias=NM[:])
            DN = sb.tile([128, 1], F32, name="DN", tag="DN")
            nc.vector.reduce_sum(DN[:], ATT[:], axis=AX.X)
            nc.vector.reciprocal(DN[:], DN[:])
            nc.vector.tensor_scalar_mul(out=ATT[:], in0=ATT[:], scalar1=DN[:])
            nc.sync.dma_start(out=o_att[:], in_=ATT[:])
            ATK = [att.tile([128, 128], F32R, name=f"ATK{t}", tag=f"ATK{t}")
                   for t in range(4)]
            for t in range(4):
                tp = transpose_f32(ATT[:, 128 * t:128 * (t + 1)])
                nc.vector.tensor_copy(ATK[t][:], tp[:])
            ctxp = psA.tile([128, 512], F32, name="ps", tag="ps")
            for t in range(4):
                nc.tensor.matmul(ctxp[:, :N_IN], ATK[t][:],
                                 VT[:, N_IN * t:N_IN * (t + 1)],
                                 start=(t == 0), stop=(t == 3))
            CTX = sb.tile([128, N_IN], F32, name="CTX", tag="CTX")
            nc.vector.tensor_copy(CTX[:], ctxp[:, :N_IN])
            tp = transpose_f32(CTX[:])
            CTN = sb.tile([N_IN, TPC], F32R, name="CTN", tag="CTN")
            nc.vector.tensor_copy(CTN[:], tp[:])
            relp = psA.tile([128, 512], F32, name="ps", tag="ps")
            nc.tensor.matmul(relp[:, :N_IN], CTN[:], WO[:], start=True, stop=True)
            RELS = sb.tile([TPC, N_IN], F32, name="RELS", tag="RELS")
            nc.vector.tensor_add(RELS[:], relp[:, :N_IN], BOB[:])
            nc.sync.dma_start(out=o_rel[:], in_=RELS[:])
            nc.vector.tensor_add(ENRO[:], RELS[:], AOWN)
            nc.sync.dma_start(out=o_enr[:], in_=ENRO[:])
            nc.sync.dma_start(out=ag1_in[:], in_=ENRO[:])

            # enriched all-gather fires while the shared/adapter path runs
            nc.gpsimd.collective_compute(
                "AllGather", ALU.bypass,
                replica_groups=[list(range(NCORE))],
                ins=[ag1_in[:].opt()], outs=[ag1_out[:].opt()],
            )

            # ---- shared = gelu(x @ Wsh + b) (own tokens) ----
            SHP = [psA.tile([128, 512], F32, name="ps", tag="ps") for _ in range(2)]
            for i in range(HT):
                wst = wsp.tile([128, H], F32R, name="ws", tag="ws")
                nc.sync.dma_start(out=wst,
                                    in_=_f32r(ws_t[128 * i:128 * (i + 1), :]))
                for h2 in range(2):
                    nc.tensor.matmul(SHP[h2][:, :384], XH[i][:, 384:512],
                                     wst[:, 384 * h2:384 * (h2 + 1)],
                                     start=(i == 0), stop=(i == HT - 1))
            SHG = sb.tile([128, H], F32, name="SHG", tag="SHG", bufs=1)
            for h2 in range(2):
                nc.vector.tensor_add(SHG[:, 384 * h2:384 * (h2 + 1)],
                                     SHP[h2][:, :384],
                                     BSH[:, 384 * h2:384 * (h2 + 1)])
            nc.scalar.activation(SHG[:], SHG[:], AF.Gelu)
            SHH = [att.tile([128, 128], F32R, name=f"SHH{i}", tag=f"SHH{i}")
                   for i in range(HT)]
            for i in range(HT):
                tp = transpose_f32(SHG[:, 128 * i:128 * (i + 1)])
                nc.vector.tensor_copy(SHH[i][:], tp[:])

            # ---- adapters: down -> scale by acts -> spec ----
            SDT = sb.tile([128, NR1], F32R, name="SDT", tag="SDT", bufs=1)
            ADT = [wsp.tile([128, NR1], F32R, name=f"adl{i}", tag=f"adl{i}",
                            bufs=1) for i in range(HT)]
            for i in range(HT):
                nc.sync.dma_start(out=ADT[i],
                                    in_=_f32r(adl[128 * i:128 * (i + 1), :]))
            for nh in range(2):
                dpp = psA.tile([128, 512], F32, name="ps", tag="ps")
                for i in range(HT):
                    nc.tensor.matmul(dpp[:], SHH[i][:],
                                     ADT[i][:, 512 * nh:512 * (nh + 1)],
                                     start=(i == 0), stop=(i == HT - 1))
                ab = ACTS[:, N_IN * 3 + 32 * nh:N_IN * 3 + 32 * (nh + 1)]
                abc = bass.AP(tensor=ab.tensor, offset=ab.offset,
                              ap=[list(ab.ap[0]), [ab.ap[1][0], 32], [0, AR]])
                sdt3 = SDT[:, 512 * nh:512 * (nh + 1)].rearrange(
                    "p (a b) -> p a b", a=32)
                dpp3 = dpp[:].rearrange("p (a b) -> p a b", a=32)
                nc.vector.tensor_tensor(out=sdt3, in0=dpp3, in1=abc, op=ALU.mult)
            SDN = [att.tile([128, 128], F32R, name=f"SDN{j}", tag=f"SDN{j}")
                   for j in range(8)]
            for j in range(8):
                t = psT.tile([128, 128], F32, name="tp", tag="tp")
                nc.tensor.transpose(t[:].bitcast(F32R),
                                    SDT[:, 128 * j:128 * (j + 1)], identr[:])
                nc.vector.tensor_copy(SDN[j][:], t[:].bitcast(F32R))
            SPP = [psA.tile([128, 512], F32, name="ps", tag="ps") for _ in range(2)]
            for j in range(8):
                upt = wsp.tile([128, H], F32R, name="upa", tag="upa")
                nc.sync.dma_start(out=upt,
                                    in_=_f32r(upa[128 * j:128 * (j + 1), :]))
                for h2 in range(2):
                    nc.tensor.matmul(SPP[h2][:, :384], SDN[j][:],
                                     upt[:, 384 * h2:384 * (h2 + 1)],
                                     start=(j == 0), stop=(j == 7))

            # ---- intermediate + LN1 -> x1 ----
            SA = sb.tile([128, 1], F32, name="SA", tag="SA")
            nc.vector.reduce_sum(SA[:], AOWN, axis=AX.X)
            XI = sb.tile([128, H], F32, name="XI", tag="XI", bufs=1)
            nc.vector.tensor_scalar_mul(out=XI[:], in0=SHG[:], scalar1=SA[:])
            for h2 in range(2):
                nc.vector.tensor_add(XI[:, 384 * h2:384 * (h2 + 1)],
                                     XI[:, 384 * h2:384 * (h2 + 1)],
                                     SPP[h2][:, :384])
            nc.vector.tensor_add(XI[:], XI[:], XB3[:])
            layernorm(sb, X1G[:], XI[:], G1B, B1B)

            # pack x1 H-major into ag2_in
            for i in range(HT):
                tp = transpose_f32(X1G[:, 128 * i:128 * (i + 1)])
                xhc = sb.tile([128, 128], F32, name="xhc", tag="xhc")
                nc.vector.tensor_copy(xhc[:], tp[:])
                nc.sync.dma_start(out=ag2_in[:, 128 * i:128 * (i + 1)], in_=xhc[:])

        # ================= ALLGATHER (x1) =================
        nc.gpsimd.collective_compute(
            "AllGather", ALU.bypass,
            replica_groups=[list(range(NCORE))],
            ins=[ag2_in[:].opt()], outs=[ag2_out[:].opt()],
        )

        # ================= PHASE C =================
        with contextlib.ExitStack() as esC:
            cp = esC.enter_context(tc.tile_pool(name="cp", bufs=2))
            x1p = esC.enter_context(tc.tile_pool(name="x1p", bufs=1))
            sdp = esC.enter_context(tc.tile_pool(name="sdp", bufs=1))
            upp = esC.enter_context(tc.tile_pool(name="upp", bufs=1))
            bc = esC.enter_context(tc.tile_pool(name="bc", bufs=2))
            psC = esC.enter_context(tc.tile_pool(name="psC", bufs=2, space="PSUM"))
            psY = esC.enter_context(tc.tile_pool(name="psY", bufs=4, space="PSUM"))

            # big weights now; DP tiles were reserved up-front
            for i in range(HT):
                nc.gpsimd.dma_start(out=DP[i],
                                    in_=_f32r(dp[128 * i:128 * (i + 1), :]))
            UP = [upp.tile([128, H], F32R, name=f"up{e}", tag=f"up{e}")
                  for e in range(EPC)]
            for e in range(EPC):
                nc.gpsimd.dma_start(out=UP[e],
                                    in_=_f32r(up[128 * e:128 * (e + 1), :]))

            PACTS = [x1p.tile([N_PROC, S], F32, name=f"PACTS{b}", tag=f"PACTS{b}")
                     for b in range(B)]
            G2B = brow(x1p, "G2B", g2, H); B2B = brow(x1p, "B2B", b2, H)

            # conv over gathered enriched -> process_acts^T per batch
            CWT = x1p.tile([N_IN, 5 * N_PROC], F32R, name="CWT", tag="CWT")
            nc.sync.dma_start(out=CWT,
                              in_=_f32r(cw[:].rearrange("n a o -> n (a o)")))
            ENT = [x1p.tile([N_IN, S + 4], F32R, name=f"ENT{b}", tag=f"ENT{b}")
                   for b in range(B)]
            ZC = x1p.tile([N_IN, 2], F32, name="ZC", tag="ZC")
            nc.vector.memset(ZC, 0.0)
            for b in range(B):
                nc.vector.tensor_copy(ENT[b][:, 0:2], ZC[:])
                nc.vector.tensor_copy(ENT[b][:, S + 2:S + 4], ZC[:])
            for c in range(NCORE):
                b, blk = c // 4, c % 4
                ec = cp.tile([TPC, N_IN], F32, name="ec", tag="ec")
                nc.sync.dma_start(out=ec, in_=ag1_out[TPC * c:TPC * (c + 1), :])
                tp = transpose_f32(ec[:])
                nc.vector.tensor_copy(
                    ENT[b][:, 2 + 128 * blk:2 + 128 * (blk + 1)], tp[:])
            for b in range(B):
                rsp = psC.tile([N_PROC, S], F32, name="pd", tag="pd")
                for ds in range(5):
                    nc.tensor.matmul(rsp[:], CWT[:, N_PROC * ds:N_PROC * (ds + 1)],
                                     ENT[b][:, ds:ds + S],
                                     start=(ds == 0), stop=(ds == 4))
                nc.scalar.activation(PACTS[b][:], rsp[:], AF.Sigmoid, bias=CBT[:])
                nc.sync.dma_start(out=o_pacts[:, S * b:S * (b + 1)],
                                  in_=PACTS[b][:])
                nc.sync.dma_start(out=pacts_d[b][:], in_=PACTS[b][:])

            # PE warm-keeper: chained f32 matmuls on x1 bridging the AG2 wait
            wup = psC.tile([128, 512], F32, name="wup", tag="pd")
            for w in range(8):
                nc.tensor.matmul(wup[:], X1G[:, 0:128], X1G[:, 0:512],
                                 start=(w == 0), stop=(w == 7))

            SD = [sdp.tile([128, S], F32R, name=f"SD{e}", tag=f"SD{e}")
                  for e in range(EPC)]
            for b in range(B):
                # x1 H-major (this batch) from the gathered payload
                X1H = [x1p.tile([128, S], F32R, name=f"X1H{i}", tag=f"X1H{i}")
                       for i in range(HT)]
                for i in range(HT):
                    for cc in range(4):
                        c = 4 * b + cc
                        nc.sync.dma_start(
                            out=X1H[i][:, TPC * cc:TPC * (cc + 1)],
                            in_=_f32r(ag2_out[TPC * c:TPC * (c + 1),
                                              128 * i:128 * (i + 1)]))
                # pass 1: down-proj + gate by process_acts
                for e in range(EPC):
                    pd = psC.tile([PR, S], F32, name="pd", tag="pd")
                    for i in range(HT):
                        nc.tensor.matmul(pd[:], DP[i][:, PR * e:PR * (e + 1)],
                                         X1H[i][:],
                                         start=(i == 0), stop=(i == HT - 1))
                    bce = bc.tile([PR, S], F32, name="bce", tag="bce")
                    nc.gpsimd.dma_start(
                        out=bce, in_=pacts_d[b][e].partition_broadcast(PR))
                    nc.vector.tensor_tensor(out=SD[e][:], in0=pd[:], in1=bce[:],
                                            op=ALU.mult)
                # pass 2: up-proj, accumulate over experts in PSUM
                for tb in range(4):
                    yp = [psY.tile([128, 384], F32, name="yp", tag="yp")
                          for _ in range(2)]
                    for e in range(EPC):
                        for h2 in range(2):
                            nc.tensor.matmul(yp[h2][:],
                                             SD[e][:, 128 * tb:128 * (tb + 1)],
                                             UP[e][:, 384 * h2:384 * (h2 + 1)],
                                             start=(e == 0), stop=(e == EPC - 1))
                    ys = cp.tile([128, H], dt.bfloat16, name="ys", tag="ys")
                    for h2 in range(2):
                        nc.vector.tensor_copy(ys[:, 384 * h2:384 * (h2 + 1)],
                                              yp[h2][:])
                    nc.sync.dma_start(
                        out=rs_in[S * b + 128 * tb:S * b + 128 * (tb + 1), :],
                        in_=ys[:])

            nc.gpsimd.collective_compute(
                "ReduceScatter", ALU.add,
                replica_groups=[list(range(NCORE))],
                ins=[rs_in[:].opt()], outs=[rs_out[:].opt()],
            )

            YRb = cp.tile([TPC, H], dt.bfloat16, name="YRb", tag="YRb", bufs=1)
            nc.sync.dma_start(out=YRb, in_=rs_out[:])
            YR = cp.tile([TPC, H], F32, name="YR", tag="YR", bufs=1)
            nc.vector.tensor_add(YR[:], YRb[:], X1G[:])
            layernorm(cp, YR[:], YR[:], G2B, B2B)
            nc.sync.dma_start(out=o_x2[:], in_=YR[:])

    nc.finalize()
    return nc


_NC_CACHE = None


def _get_nc():
    global _NC_CACHE
    if _NC_CACHE is None:
        _NC_CACHE = build_kernel()
    return _NC_CACHE


def _prep_inputs(inputs):
    f = lambda k: np.asarray(inputs[k], dtype=np.float32)
    x = f("x"); patterns = f("patterns")
    w_shared = f("w_shared"); b_shared = f("b_shared")
    adapt_down = f("adapt_down"); adapt_up = f("adapt_up")
    g1 = f("g1"); b1 = f("b1"); g2 = f("g2"); b2 = f("b2")
    in_proj_w = f("in_proj_w"); in_proj_b = f("in_proj_b")
    out_proj_w = f("out_proj_w"); out_proj_b = f("out_proj_b")
    conv_w = f("conv_w"); conv_b = f("conv_b")
    down_proj = f("down_proj"); up_proj = f("up_proj")

    adl = np.ascontiguousarray(adapt_down.transpose(1, 0, 2).reshape(H, NR1))
    upa = np.ascontiguousarray(adapt_up.reshape(NR1, H))
    ws_t = np.ascontiguousarray(w_shared.T)  # 'bsh,oh->bso': rows of w are o
    wq_t = np.ascontiguousarray(in_proj_w[0:N_IN].T)
    wk_t = np.ascontiguousarray(in_proj_w[N_IN:2 * N_IN].T)
    wv_t = np.ascontiguousarray(in_proj_w[2 * N_IN:3 * N_IN].T)
    wo_t = np.ascontiguousarray(out_proj_w.T)
    bq = in_proj_b[0:N_IN]; bk = in_proj_b[N_IN:2 * N_IN]
    bv = in_proj_b[2 * N_IN:3 * N_IN]
    cwm = conv_w[:, 0].transpose(2, 1, 0)  # [o,ds,n] -> [n,ds,o]

    in_maps, metas = [], []
    for c in range(NCORE):
        b, s0 = c // 4, (c % 4) * TPC
        R = 384 - s0
        x_roll = np.ascontiguousarray(np.roll(x[b], R, axis=0))
        x_roll_t = np.ascontiguousarray(x_roll.T)
        kk = np.arange(S)
        s_k = (kk - R) % S
        s_q = s0 + np.arange(TPC)
        mskv = np.where(s_k[None, :] <= s_q[:, None], 0.0,
                        NEG_INF).astype(np.float32)
        perm = np.r_[np.arange(EPC * c, EPC * (c + 1)),
                     [i for i in range(N_PROC)
                      if not (EPC * c <= i < EPC * (c + 1))]].astype(np.int64)
        dp_c = np.ascontiguousarray(
            down_proj[EPC * c:EPC * (c + 1)].transpose(1, 0, 2).reshape(H, NR2))
        up_c = np.ascontiguousarray(
            up_proj[EPC * c:EPC * (c + 1)].reshape(NR2, H))
        in_maps.append(dict(
            x_roll=x_roll, x_roll_t=x_roll_t, msk=mskv, patterns=patterns, ws_t=ws_t,
            b_shared=b_shared, adl=adl, upa=upa, g1=g1, b1=b1, g2=g2, b2=b2,
            wq_t=wq_t, wk_t=wk_t, wv_t=wv_t, wo_t=wo_t,
            bq=bq, bk=bk, bv=bv, bo=out_proj_b,
            cw=np.ascontiguousarray(cwm[:, :, perm]),
            cb=np.ascontiguousarray(conv_b[perm]),
            dp=dp_c, up=up_c,
        ))
        metas.append(dict(R=R, perm=perm))
    return in_maps, metas


def kernel(**inputs):
    nc = _get_nc()
    in_maps, metas = _prep_inputs(inputs)
    res = run_bass_kernel_spmd(nc, in_maps, list(range(NCORE)))
    rs = res.results

    x2 = np.concatenate([rs[c]["o_x2"] for c in range(NCORE)]).reshape(B, S, H)
    ia = np.concatenate([rs[c]["o_ia"] for c in range(NCORE)]).reshape(B, S, N_IN)
    rel = np.concatenate([rs[c]["o_rel"] for c in range(NCORE)]).reshape(B, S, N_IN)
    enr = np.concatenate([rs[c]["o_enr"] for c in range(NCORE)]).reshape(B, S, N_IN)
    att = np.zeros((B, S, S), np.float32)
    for c in range(NCORE):
        b, s0 = c // 4, (c % 4) * TPC
        att[b, s0:s0 + TPC] = np.roll(rs[c]["o_att"], -metas[c]["R"], axis=1)
    pact = np.zeros((B, S, N_PROC), np.float32)
    pa_all = rs[0]["o_pacts"]  # [N_PROC (perm of core0), B*S]
    inv = np.empty(N_PROC, np.int64)
    inv[metas[0]["perm"]] = np.arange(N_PROC)
    for b in range(B):
        pact[b] = pa_all[:, S * b:S * (b + 1)][inv].T
    return x2, ia, rel, enr, pact, att
